# revision 1
# baseline (speedup 1.0000x reference)
"""Trainium2 Bass kernel for nn_Entropy (histogram_binning): per-pixel Shannon
entropy of a 5x5-window KDE histogram over 256 intensity bins.

Math (validated in f32 vs reference):
  k(x,b) = sigmoid'(10(x-b)) = 0.25*(1 - tanh^2(5x-5b))   [exact identity]
  q[h,w,b] = 5x5 window sum of k;  S = sum_b q;  p = q/(S+EPS)
  out = -sum_b p*ln(p+EPS) = -r * sum_b q*ln(r*q+EPS),  r = 1/(S+EPS)
  S comes analytically per pixel from 5 taps of the KDE kernel around
  frac(x) (range-masked), then a tiny 5x5 window sum.

Layout per (image, bin-half) stripe: partitions = h (96), free = (w, b).
  - d' = 5x - 5b on TensorE: K=9 matmuls; stationary = [5*x^T(8 w-rows);
    ones], moving = tiny shipped selector constant.
  - tanh on ScalarE (evacuates PSUM); k = 0.25 - 0.25 t^2 on VectorE.
  - H-window: banded-matrix matmul (TensorE) -> PSUM, evacuated by
    ScalarE into a w-inner padded stripe [b-block: 3 zero pads + 96 w].
  - W-window: one in-place VectorE prefix scan per stripe over the padded
    row; q[w,b] = P[99b+w+5] - P[99b+w] (pads absorb all edges).
  - backend per w: L = ln(r*q + EPS) on ScalarE (per-partition scale AP),
    e = q*L and QL = sum_b(e) on VectorE; E = -r*QL.

Stripes are software-pipelined (3 stripe buffers) so image i+1's front end
overlaps image i's backend. Sharding: B*C = 24 images, 3 per core across 8
cores; no collectives. Self-contained; compiled once per process.
"""

import sys

sys.path.insert(0, "/opt/trn_rl_repo")

import numpy as np

H = 96
W = 96
NB = 256
NBH = 128         # bins per stripe (half)
NIMG = 3
NCORES = 8
EPS = 1e-10
ZB = 99           # per-bin block in a stripe: 3 zero pads + 96 w cols
WQ = 8            # w rows per stationary group
NG = W // WQ      # 12 groups

_CACHE = {}


def _build_consts():
    # selector constants per bin-half: [9, WQ*NBH]; rows j=0..7 mark w-offset
    # j over that bin-block; row 8 = -5*b
    crhs = []
    for half in range(2):
        c = np.zeros((9, WQ * NBH), dtype=np.float32)
        for j in range(WQ):
            c[j, j * NBH:(j + 1) * NBH] = 1.0
        b = np.arange(NBH, dtype=np.float32) + half * NBH
        c[8, :] = np.tile(-5.0 * b, WQ)
        crhs.append(c)
    hh = np.arange(H)
    band = (np.abs(hh[:, None] - hh[None, :]) <= 2).astype(np.float32)
    return crhs[0], crhs[1], band


def _emit_kernel(nc, tc, ctx, ins, outs):
    from concourse import mybir

    f32 = mybir.dt.float32
    i32 = mybir.dt.int32
    AF = mybir.ActivationFunctionType
    OP = mybir.AluOpType

    x_d, xt_d, crhs0_d, crhs1_d, band_d = ins
    (ent_d,) = outs
    NW = NIMG * W

    consts = ctx.enter_context(tc.tile_pool(name="consts", bufs=1))
    stripes = ctx.enter_context(tc.tile_pool(name="stripes", bufs=2))
    sm = ctx.enter_context(tc.tile_pool(name="sm", bufs=1))
    chunks = ctx.enter_context(tc.tile_pool(name="chunks", bufs=2))
    psum = ctx.enter_context(tc.tile_pool(name="psum", bufs=4, space="PSUM"))

    # ---- constants / inputs ----
    crhs_sb = []
    for half, cd in ((0, crhs0_d), (1, crhs1_d)):
        t = consts.tile([73, WQ * NBH], f32, tag=f"crhs{half}")
        for k3 in range(3):
            nc.sync.dma_start(t[32 * k3:32 * k3 + 9, :], cd[:])
        crhs_sb.append(t)
    band_sb = consts.tile([H, H], f32)
    nc.sync.dma_start(band_sb[:], band_d[:])

    xall = consts.tile([H, NW], f32)
    xtall = consts.tile([W, NIMG * H], f32)
    for i in range(NIMG):
        nc.sync.dma_start(xall[:, i * W:(i + 1) * W], x_d[i])
        nc.sync.dma_start(xtall[:, i * H:(i + 1) * H], xt_d[i])

    ones_sb = consts.tile([1, NIMG * H], f32)
    nc.vector.memset(ones_sb[:], 1.0)
    xt5_all = consts.tile([W, NIMG * H], f32)
    nc.vector.tensor_scalar(xt5_all[:], xtall[:], 5.0, None, op0=OP.mult)
    # stationary groups [9 rows: 5*xT(8 w) ; ones], 3 per tile at bases 0/32/64
    xt9g = []
    for tg in range(4):
        gt = consts.tile([73, NIMG * H], f32, tag=f"xt9g{tg}")
        for k3 in range(3):
            g = tg * 3 + k3
            base = 32 * k3
            nc.sync.dma_start(gt[base:base + 8, :], xt5_all[8 * g:8 * g + 8, :])
            nc.sync.dma_start(gt[base + 8:base + 9, :], ones_sb[:])
            xt9g.append(gt[base:base + 9])

    bias_tiles = {}

    def bias_ap(val):
        if val not in bias_tiles:
            t = consts.tile([H, 1], f32, tag=f"bias{val}")
            nc.vector.memset(t[:], val)
            bias_tiles[val] = t
        return bias_tiles[val][:]

    # =====================  S path (tiny, [96, 288])  =====================
    ni = sm.tile([H, NW], i32)
    nc.vector.tensor_copy(ni[:], xall[:])
    nf = sm.tile([H, NW], f32)
    nc.vector.tensor_copy(nf[:], ni[:])
    u = sm.tile([H, NW], f32)
    nc.vector.tensor_tensor(u[:], xall[:], nf[:], op=OP.subtract)
    taps = (-2, -1, 0, 1, 2)
    sq = {}
    for o in taps:
        v = sm.tile([H, NW], f32, tag=f"v{o}")
        nc.scalar.activation(v[:], u[:], AF.Tanh, bias=bias_ap(-5.0 * o), scale=5.0)
        s2 = sm.tile([H, NW], f32, tag=f"sq{o}")
        nc.scalar.activation(s2[:], v[:], AF.Square)
        sq[o] = s2
    masks = {}
    for o in taps:
        if o == 0:
            continue
        m = sm.tile([H, NW], f32, tag=f"m{o}")
        if o < 0:
            nc.vector.tensor_scalar(m[:], nf[:], float(-o), None, op0=OP.is_ge)
        else:
            nc.vector.tensor_scalar(m[:], nf[:], float(255 - o), None, op0=OP.is_le)
        masks[o] = m
    cnt = sm.tile([H, NW], f32)
    nc.vector.tensor_tensor(cnt[:], masks[-2][:], masks[-1][:], op=OP.add)
    nc.vector.tensor_tensor(cnt[:], cnt[:], masks[1][:], op=OP.add)
    nc.vector.tensor_tensor(cnt[:], cnt[:], masks[2][:], op=OP.add)
    nc.vector.tensor_scalar(cnt[:], cnt[:], 1.0, None, op0=OP.add)
    ssum = sm.tile([H, NW], f32)
    nc.vector.tensor_copy(ssum[:], sq[0][:])
    for o in (-2, -1, 1, 2):
        t_m = sm.tile([H, NW], f32, tag=f"tm{o}")
        nc.vector.tensor_tensor(t_m[:], masks[o][:], sq[o][:], op=OP.mult)
        nc.vector.tensor_tensor(ssum[:], ssum[:], t_m[:], op=OP.add)
    spix = sm.tile([H, NW], f32)
    nc.vector.tensor_tensor(spix[:], cnt[:], ssum[:], op=OP.subtract)
    nc.vector.tensor_scalar(spix[:], spix[:], 0.25, None, op0=OP.mult)
    ps_s = psum.tile([H, 1024], f32, tag="ps")
    nc.tensor.matmul(ps_s[:, 0:NW], band_sb[:], spix[:], start=True, stop=True)
    sh = sm.tile([H, NW], f32)
    nc.scalar.copy(sh[:], ps_s[:, 0:NW])
    shp = sm.tile([H, NIMG, W + 4], f32)
    nc.vector.memset(shp[:], 0.0)
    for i in range(NIMG):
        nc.vector.tensor_copy(shp[:, i, 2:2 + W], sh[:, i * W:(i + 1) * W])
    swin = sm.tile([H, NIMG, W], f32)
    nc.vector.tensor_tensor(swin[:], shp[:, :, 0:W], shp[:, :, 1:1 + W], op=OP.add)
    for j in (2, 3, 4):
        nc.vector.tensor_tensor(swin[:], swin[:], shp[:, :, j:j + W], op=OP.add)
    rtile = sm.tile([H, NW], f32)
    sw_flat = swin[:].rearrange("p a b -> p (a b)")
    nc.vector.tensor_scalar(rtile[:], sw_flat, EPS, None, op0=OP.add)
    nc.vector.reciprocal(rtile[:], rtile[:])

    # =====================  main path: per (image, bin-half) stripe  ========
    QL = sm.tile([H, NW], f32)
    stripe_store = {}

    def emit_front(i, half):
        qh = stripes.tile([H, NBH * ZB + 8], f32, tag="qh")
        qh3 = qh[:, 0:NBH * ZB].rearrange("p (b z) -> p b z", z=ZB)
        nc.vector.memset(qh3[:, :, 0:3], 0.0)
        nc.vector.memset(qh[:, NBH * ZB:], 0.0)

        for c in range(NG // 2):  # chunks of 2 w-groups = [96, 2048] cols
            pd = psum.tile([H, 1024], f32, tag="ps")
            pd2 = psum.tile([H, 1024], f32, tag="ps")
            for piece, pt in ((0, pd), (1, pd2)):
                g = 2 * c + piece
                base = 32 * (g % 3)
                nc.tensor.matmul(
                    pt[:, 0:512],
                    xt9g[g][:, i * H:(i + 1) * H],
                    crhs_sb[half][base:base + 9, 0:512],
                    start=True, stop=True,
                )
                nc.tensor.matmul(
                    pt[:, 512:1024],
                    xt9g[g][:, i * H:(i + 1) * H],
                    crhs_sb[half][base:base + 9, 512:1024],
                    start=True, stop=True,
                )
            tt = chunks.tile([H, 2048], f32, tag="t")
            nc.scalar.activation(tt[:, 0:1024], pd[:], AF.Tanh)
            nc.scalar.activation(tt[:, 1024:2048], pd2[:], AF.Tanh)
            kk = chunks.tile([H, 2048], f32, tag="k")
            nc.vector.tensor_tensor(kk[:], tt[:], tt[:], op=OP.mult)
            nc.vector.tensor_scalar(kk[:], kk[:], -0.25, 0.25, op0=OP.mult, op1=OP.add)
            for piece in range(2):
                ph = psum.tile([H, 1024], f32, tag="ps")
                for pp in range(2):
                    nc.tensor.matmul(
                        ph[:, pp * 512:(pp + 1) * 512],
                        band_sb[:],
                        kk[:, piece * 1024 + pp * 512:piece * 1024 + (pp + 1) * 512],
                        start=True, stop=True,
                    )
                # evac: chunk piece covers w-group g = 2c+piece (8 w), all bins
                g = 2 * c + piece
                dst = qh3[:, :, 3 + 8 * g:3 + 8 * g + 8].transpose([0, 2, 1])
                nc.scalar.copy(dst, ph[:].rearrange("p (w b) -> p w b", b=NBH))

        nc.vector.tensor_tensor_scan(
            qh[:], qh[:], qh[:], 0.0, op0=OP.add, op1=OP.bypass
        )
        stripe_store[(i, half)] = (qh, qh3)

    def emit_backend(i):
        qhs = [stripe_store.pop((i, 0)), stripe_store.pop((i, 1))]
        for wc in range(W // 4):
            w0 = 4 * wc
            qt = chunks.tile([H, 4, NB], f32, tag="q")
            for half, (qh, qh3) in enumerate(qhs):
                if w0 + 9 <= ZB:
                    hi = qh3[:, :, w0 + 5:w0 + 9].transpose([0, 2, 1])
                    lo = qh3[:, :, w0:w0 + 4].transpose([0, 2, 1])
                    nc.vector.tensor_tensor(
                        qt[:, :, half * NBH:(half + 1) * NBH], hi, lo,
                        op=OP.subtract,
                    )
                else:
                    for wi in range(4):
                        nc.vector.tensor_tensor(
                            qt[:, wi, half * NBH:(half + 1) * NBH],
                            qh[:, w0 + 5 + wi::ZB][:, 0:NBH],
                            qh[:, w0 + wi::ZB][:, 0:NBH],
                            op=OP.subtract,
                        )
            ltile = chunks.tile([H, 1024], f32, tag="L")
            for j in range(4):
                w = w0 + j
                rcol = rtile[:, i * W + w:i * W + w + 1]
                nc.scalar.activation(
                    ltile[:, j * 256:(j + 1) * 256],
                    qt[:, j, :],
                    AF.Ln,
                    bias=bias_ap(EPS),
                    scale=rcol,
                )
            l3 = ltile[:].rearrange("p (a b) -> p a b", b=NB)
            nc.vector.tensor_tensor(l3, qt[:], l3, op=OP.mult)
            nc.vector.tensor_reduce(
                QL[:, i * W + w0:i * W + w0 + 4],
                l3,
                axis=mybir.AxisListType.X,
                op=OP.add,
            )

    emit_front(0, 0)
    emit_front(0, 1)
    emit_front(1, 0)
    emit_backend(0)
    emit_front(1, 1)
    emit_front(2, 0)
    emit_backend(1)
    emit_front(2, 1)
    emit_backend(2)

    # E = -(r * QL) ; write out
    ent = sm.tile([H, NW], f32)
    nc.vector.tensor_tensor(ent[:], rtile[:], QL[:], op=OP.mult)
    nc.vector.tensor_scalar(ent[:], ent[:], -1.0, None, op0=OP.mult)
    for i in range(NIMG):
        nc.sync.dma_start(ent_d[i], ent[:, i * W:(i + 1) * W])


def _get_compiled():
    if "nc" in _CACHE:
        return _CACHE["nc"]
    from contextlib import ExitStack

    import concourse.tile as tile
    from concourse import bacc, mybir

    f32 = mybir.dt.float32
    nc = bacc.Bacc("TRN2", target_bir_lowering=False, debug=False)
    x_d = nc.dram_tensor("x_sh", [NIMG, H, W], f32, kind="ExternalInput").ap()
    xt_d = nc.dram_tensor("xt_sh", [NIMG, W, H], f32, kind="ExternalInput").ap()
    crhs0_d = nc.dram_tensor("crhs0", [9, WQ * NBH], f32, kind="ExternalInput").ap()
    crhs1_d = nc.dram_tensor("crhs1", [9, WQ * NBH], f32, kind="ExternalInput").ap()
    band_d = nc.dram_tensor("bandh", [H, H], f32, kind="ExternalInput").ap()
    ent_d = nc.dram_tensor("ent", [NIMG, H, W], f32, kind="ExternalOutput").ap()

    with tile.TileContext(nc) as tc:
        with ExitStack() as ctx:
            _emit_kernel(
                nc, tc, ctx, (x_d, xt_d, crhs0_d, crhs1_d, band_d), (ent_d,)
            )
    nc.compile()
    _CACHE["nc"] = nc
    return nc


def make_in_maps(x):
    """x: full [8, 3, 96, 96] -> list of 8 per-core input dicts."""
    x = np.ascontiguousarray(np.asarray(x, dtype=np.float32))
    imgs = x.reshape(NCORES * NIMG, H, W)
    crhs0, crhs1, band = _build_consts()
    in_maps = []
    for c in range(NCORES):
        sh = np.ascontiguousarray(imgs[c * NIMG:(c + 1) * NIMG])
        in_maps.append(
            {
                "x_sh": sh,
                "xt_sh": np.ascontiguousarray(sh.transpose(0, 2, 1)),
                "crhs0": crhs0,
                "crhs1": crhs1,
                "bandh": band,
            }
        )
    return in_maps


def kernel(x):
    """Full inputs in, full outputs out. x: [8, 3, 96, 96] f32."""
    from concourse.bass_utils import run_bass_kernel_spmd

    nc = _get_compiled()
    in_maps = make_in_maps(x)
    res = run_bass_kernel_spmd(nc, in_maps, list(range(NCORES)))
    out = np.stack([res.results[c]["ent"] for c in range(NCORES)])
    return out.reshape(8, 3, H, W).astype(np.float32)



# revision 6
# speedup vs baseline: 1.5559x; 1.5559x over previous
"""Trainium2 Bass kernel for nn_Entropy (histogram_binning): per-pixel Shannon
entropy of a 5x5-window KDE histogram over 256 intensity bins.

v2 design (Exp-based front end, single activation table, custom DVE ops):
  k(x,b) = sig'(10(x-b)) = u/(1+u)^2 with u = exp(-10|x-b|).
  Layout per (image): h on partitions, free = (bin-block, w-inner) with
  4 zero-pads per 100-col block (plus 5 lead / 2 trail pads per half-chunk).
  Pipeline per superchunk (8 bins x 96 w = 768 cols, 16 sc per half-pair):
    TensorE  D = x - b          (K=97 matmul: stationary [x^T; 1], moving
                                 delta-selector + (-b) row, fp32)
    DVE      a = |D - 128h|     (custom ABSDEV, fp16, strided into padded
                                 chunk; pads preset to 1e4 so exp(pad) = 0)
    ScalarE  u = Exp(-10 a)     (fp16)
    DVE      v = u + u^2(c2+c3u) (custom VPOLY ~ u/(1+u), rel err ~0.5%)
    DVE      A5 = running 5-window of k = v(1-v)  (custom WIN5K scan op:
             state += k(v[i]) - k(v[i-5]); block pads make all SAME edges
             exact with zero fixups)
    TensorE  P = band @ A5      (fp16 H-window band matmul -> PSUM = q)
    ScalarE  L = Ln(q + 2e-6)   (batched; eps-shift identity removes the
                                 per-pixel 1/S scale from the log argument)
    DVE      e = q * L          (PSUM x fp16 -> e-stripe fp16)
  Per half: 7-level pairwise tree-reduce of e over bins -> T = sum_b q ln q.
  Analytic S-path (5 taps of the same fp16 Exp/poly chain on [96,288]) gives
  S = sum_b q; E = ln(S+EPS) - T/(S+EPS).  Exp/Ln/Identity/Abs share one
  activation table -> no ACT table reloads.
  Sharding: B*C = 24 images, 3 per core across 8 cores; no collectives.
"""

import sys

sys.path.insert(0, "/opt/trn_rl_repo")

import numpy as np

H = 96
W = 96
NBH = 128          # bins per half
NIMG = 3
NCORES = 8
EPS = 1e-10
EPS1 = 2e-6
C2P, C3P = -0.89877895, 0.43582129   # v-poly coeffs: v = u + u^2(C2P + C3P u)
BLK = 100          # per-bin block: 4 pads + 96 w
NSC = 16           # superchunks per image (8 bins each, both halves)
BPS = 8            # bins per superchunk per half
HCH = 5 + BPS * BLK + 2   # half-chunk cols: 5 lead + 800 + 2 trail = 807
PAD = 1e4          # a-pad value: exp(-10*PAD) == 0

_CACHE = {}


def _register_dve_ops():
    import concourse.dve_ops as dve_ops
    from concourse.dve_ops import DveOp
    from concourse.dve_spec import (
        C0, C1, AluOp, One, Spec, Src0, Src1, maxx, scan, sq,
    )

    def register(op):
        if op.name not in dve_ops._SUB_OPCODE_FOR_NAME:
            dve_ops.OPS.append(op)
            dve_ops._SUB_OPCODE_FOR_NAME[op.name] = (
                dve_ops._CUSTOM_DVE_ROW_BASE + len(dve_ops.OPS) - 1
            )
        else:
            op = next(o for o in dve_ops.OPS if o.name == op.name)
        return op

    absdev = register(DveOp(
        "ABSDEV_ANT",
        Spec(body=maxx(Src0 - C0, C0 - Src0),
             reference=lambda in0, in1, c0, c1, c2: np.abs(
                 in0.astype(np.float32) - c0).astype(np.float32)),
        subdim=False,
        uops_sha={"v3": "a5866c869c7d6e30", "v4": "006fe4b232e6035a"}))

    vpoly = register(DveOp(
        "VPOLY_ANT",
        Spec(body=Src0 + sq(Src0) * (C0 + C1 * Src0),
             reference=lambda in0, in1, c0, c1, c2: (
                 lambda u: (u + u * u * (c0 + c1 * u)).astype(np.float32)
             )(in0.astype(np.float32))),
        subdim=False,
        uops_sha={"v3": "217961e937d92645", "v4": "56741f276e7f1259"}))

    win5 = register(DveOp(
        "WIN5K_ANT",
        Spec(body=scan(AluOp.ADD, Src0 * (One - Src0) - Src1 * (One - Src1)),
             reference=lambda in0, in1, c0, c1, c2: np.cumsum(
                 in0.astype(np.float32) * (1 - in0.astype(np.float32))
                 - in1.astype(np.float32) * (1 - in1.astype(np.float32)),
                 axis=-1, dtype=np.float32)),
        subdim=False,
        uops_sha={"v3": "9d91f28b1ae18abb", "v4": "1425a9f273284709"}))

    return absdev, vpoly, win5


def _build_consts():
    # selector moving operand [97, 128*96]: col c = b_local*96 + w.
    # rows k<96: delta(k == w); row 96: -b_local.
    sel = np.zeros((97, NBH * W), dtype=np.float32)
    cols = np.arange(NBH * W)
    bl = cols // W
    w = cols % W
    sel[w, cols] = 1.0
    sel[96, :] = -bl.astype(np.float32)
    hh = np.arange(H)
    band = (np.abs(hh[:, None] - hh[None, :]) <= 2).astype(np.float16)
    return sel, band


def _emit_kernel(nc, tc, ctx, ins, outs, ops):
    from concourse import mybir

    f32 = mybir.dt.float32
    f16 = mybir.dt.float16
    i32 = mybir.dt.int32
    AF = mybir.ActivationFunctionType
    OP = mybir.AluOpType

    ABSDEV, VPOLY, WIN5 = ops
    x_d, xt_d, sel_d, band_d = ins
    (ent_d,) = outs
    NW = NIMG * W

    consts = ctx.enter_context(tc.tile_pool(name="consts", bufs=1))
    sm = ctx.enter_context(tc.tile_pool(name="sm", bufs=1))
    apool = ctx.enter_context(tc.tile_pool(name="ap", bufs=2))
    upool = ctx.enter_context(tc.tile_pool(name="up", bufs=2))
    vpool = ctx.enter_context(tc.tile_pool(name="vp", bufs=2))
    a5pool = ctx.enter_context(tc.tile_pool(name="a5p", bufs=2))
    lpool = ctx.enter_context(tc.tile_pool(name="lp", bufs=2))
    epool = ctx.enter_context(tc.tile_pool(name="ep", bufs=1))
    tpool = ctx.enter_context(tc.tile_pool(name="tp", bufs=1))
    dpsum = ctx.enter_context(tc.tile_pool(name="dps", bufs=2, space="PSUM"))
    ppsum = ctx.enter_context(tc.tile_pool(name="pps", bufs=1, space="PSUM"))

    # ---- constants / inputs ----
    sel_sb = consts.tile([97, NBH * W], f32)
    nc.sync.dma_start(sel_sb[:], sel_d[:])
    band_sb = consts.tile([H, H], f16)
    nc.sync.dma_start(band_sb[:], band_d[:])

    xall = consts.tile([H, NW], f32)
    st = consts.tile([97, NIMG * H], f32)
    for i in range(NIMG):
        nc.sync.dma_start(xall[:, i * W:(i + 1) * W], x_d[i])
        nc.sync.dma_start(st[0:96, i * H:(i + 1) * H], xt_d[i])
    nc.vector.memset(st[96:97, :], 1.0)

    bias_tiles = {}

    def bias_ap(val):
        if val not in bias_tiles:
            t = consts.tile([H, 1], f32, tag=f"bias{val}")
            nc.vector.memset(t[:], val)
            bias_tiles[val] = t
        return bias_tiles[val][:]

    # =====================  S path ([96, 288])  =====================
    ni = sm.tile([H, NW], i32)
    nc.vector.tensor_copy(ni[:], xall[:])
    nf = sm.tile([H, NW], f32)
    nc.vector.tensor_copy(nf[:], ni[:])
    ufrac = sm.tile([H, NW], f32)
    nc.vector.tensor_tensor(ufrac[:], xall[:], nf[:], op=OP.subtract)
    taps = (-2, -1, 0, 1, 2)
    atap = sm.tile([H, 5, NW], f16)
    for oi, o in enumerate(taps):
        # a_o = |ufrac - o|; ufrac in [-0.5, 0.5] (i32 copy rounds to nearest)
        nc.vector._custom_dve(
            ABSDEV, out=atap[:, oi, :], in0=ufrac[:], s0=float(o))
    utap = sm.tile([H, 5, NW], f16)
    nc.scalar.activation(utap[:], atap[:], AF.Exp, scale=-10.0)
    vtap = sm.tile([H, 5, NW], f16)
    nc.vector._custom_dve(VPOLY, out=vtap[:], in0=utap[:], s0=C2P, s1=C3P)
    vsq = sm.tile([H, 5, NW], f16)
    nc.vector.tensor_tensor(vsq[:], vtap[:], vtap[:], op=OP.mult)
    ktap = sm.tile([H, 5, NW], f16)
    nc.vector.tensor_tensor(ktap[:], vtap[:], vsq[:], op=OP.subtract)

    spix = sm.tile([H, NW], f32)
    nc.vector.tensor_copy(spix[:], ktap[:, 2, :])  # o=0 tap, always valid
    for oi, o in enumerate(taps):
        if o == 0:
            continue
        m = sm.tile([H, NW], f32, tag=f"m{o}")
        if o < 0:
            nc.vector.tensor_scalar(m[:], nf[:], float(-o), None, op0=OP.is_ge)
        else:
            nc.vector.tensor_scalar(
                m[:], nf[:], float(255 - o), None, op0=OP.is_le)
        tm = sm.tile([H, NW], f32, tag=f"tm{o}")
        nc.vector.tensor_tensor(tm[:], m[:], ktap[:, oi, :], op=OP.mult)
        nc.vector.tensor_tensor(spix[:], spix[:], tm[:], op=OP.add)
    spix16 = sm.tile([H, NW], f16)
    nc.vector.tensor_copy(spix16[:], spix[:])

    ps_s = ppsum.tile([H, 2048], f32, tag="pp")
    nc.tensor.matmul(ps_s[:, 0:NW], band_sb[:], spix16[:], start=True, stop=True)
    sh = sm.tile([H, NW], f32)
    nc.scalar.copy(sh[:], ps_s[:, 0:NW])
    shp = sm.tile([H, NIMG, W + 4], f32)
    nc.vector.memset(shp[:], 0.0)
    for i in range(NIMG):
        nc.vector.tensor_copy(shp[:, i, 2:2 + W], sh[:, i * W:(i + 1) * W])
    swin = sm.tile([H, NIMG, W], f32)
    nc.vector.tensor_tensor(swin[:], shp[:, :, 0:W], shp[:, :, 1:1 + W], op=OP.add)
    for j in (2, 3, 4):
        nc.vector.tensor_tensor(swin[:], swin[:], shp[:, :, j:j + W], op=OP.add)
    sw_flat = swin[:].rearrange("p a b -> p (a b)")
    rtile = sm.tile([H, NW], f32)
    nc.vector.tensor_scalar(rtile[:], sw_flat, EPS, None, op0=OP.add)
    nc.vector.reciprocal(rtile[:], rtile[:])
    lns = sm.tile([H, NW], f32)
    nc.scalar.activation(lns[:], sw_flat, AF.Ln, bias=bias_ap(EPS))

    # =====================  main path  =====================
    QL = sm.tile([H, NW], f32)
    estripes = {}
    for h in range(2):
        est = epool.tile([H, NBH * W], f16, tag=f"e{h}")
        estripes[h] = est
    # tree scratch
    ts1 = tpool.tile([H, NBH * W // 2], f16)
    ts2 = tpool.tile([H, NBH * W // 4], f16)

    npads_set = [0]

    for i in range(NIMG):
        for sc in range(NSC):
            dt = dpsum.tile([H, 768], f32, tag="d")
            mvbase = 768 * sc
            nc.tensor.matmul(
                dt[:, 0:512], st[:, i * H:(i + 1) * H],
                sel_sb[:, mvbase:mvbase + 512], start=True, stop=True)
            nc.tensor.matmul(
                dt[:, 512:768], st[:, i * H:(i + 1) * H],
                sel_sb[:, mvbase + 512:mvbase + 768], start=True, stop=True)

            at = apool.tile([H, 2 * HCH], f16, tag="a")
            if npads_set[0] < 2:
                nc.vector.memset(at[:], PAD)
                npads_set[0] += 1
            for h in range(2):
                dst = at[:, h * HCH + 5:h * HCH + 5 + BPS * BLK] \
                    .rearrange("p (b z) -> p b z", z=BLK)[:, :, 4:BLK]
                nc.vector._custom_dve(
                    ABSDEV, out=dst, in0=dt[:, 0:768], s0=128.0 * h)
            ut = upool.tile([H, 2 * HCH], f16, tag="u")
            nc.scalar.activation(ut[:], at[:], AF.Exp, scale=-10.0)
            vt = vpool.tile([H, 2 * HCH], f16, tag="v")
            nc.vector._custom_dve(VPOLY, out=vt[:], in0=ut[:], s0=C2P, s1=C3P)
            a5 = a5pool.tile([H, 2 * HCH], f16, tag="a5")
            nc.vector._custom_dve(
                WIN5, out=a5[:, 0:2 * HCH - 5], in0=vt[:, 5:2 * HCH],
                in1=vt[:, 0:2 * HCH - 5])

            pt = ppsum.tile([H, 2048], f32, tag="pp")
            for h in range(2):
                for pp in range(2):
                    off = h * HCH + 400 * pp + 6
                    mvap = a5[:, off:off + 400] \
                        .rearrange("p (b z) -> p b z", z=BLK)[:, :, 0:96]
                    nc.tensor.matmul(
                        pt[:, 1024 * h + 512 * pp:1024 * h + 512 * pp + 384],
                        band_sb[:], mvap, start=True, stop=True)
            lt = lpool.tile([H, 1536], f16, tag="l")
            nc.scalar.activation(
                lt[:].rearrange("p (a b) -> p a b", b=384),
                pt[:].rearrange("p (a b) -> p a b", b=512)[:, :, 0:384],
                AF.Ln, bias=bias_ap(EPS1))
            for h in range(2):
                nc.vector.tensor_tensor(
                    estripes[h][:, 768 * sc:768 * sc + 768]
                        .rearrange("p (a b) -> p a b", b=384),
                    pt[:, 1024 * h:1024 * h + 1024]
                        .rearrange("p (a b) -> p a b", b=512)[:, :, 0:384],
                    lt[:, 768 * h:768 * h + 768]
                        .rearrange("p (a b) -> p a b", b=384),
                    op=OP.mult)

        # tree reduce per half -> T = sum_b e
        th = {}
        for h in range(2):
            e = estripes[h]
            nc.vector.tensor_tensor(
                ts1[:, 0:6144], e[:, 0:6144], e[:, 6144:12288], op=OP.add)
            nc.vector.tensor_tensor(
                ts2[:, 0:3072], ts1[:, 0:3072], ts1[:, 3072:6144], op=OP.add)
            nc.vector.tensor_tensor(
                ts1[:, 0:1536], ts2[:, 0:1536], ts2[:, 1536:3072], op=OP.add)
            nc.vector.tensor_tensor(
                ts2[:, 0:768], ts1[:, 0:768], ts1[:, 768:1536], op=OP.add)
            nc.vector.tensor_tensor(
                ts1[:, 0:384], ts2[:, 0:384], ts2[:, 384:768], op=OP.add)
            nc.vector.tensor_tensor(
                ts2[:, 0:192], ts1[:, 0:192], ts1[:, 192:384], op=OP.add)
            tt = sm.tile([H, W], f32, tag=f"th{h}")
            nc.vector.tensor_tensor(
                tt[:], ts2[:, 0:96], ts2[:, 96:192], op=OP.add)
            th[h] = tt
        nc.vector.tensor_tensor(
            QL[:, i * W:(i + 1) * W], th[0][:], th[1][:], op=OP.add)

    # E = lnS - r*T
    ent = sm.tile([H, NW], f32)
    nc.vector.tensor_tensor(ent[:], rtile[:], QL[:], op=OP.mult)
    nc.vector.tensor_tensor(ent[:], lns[:], ent[:], op=OP.subtract)
    for i in range(NIMG):
        nc.sync.dma_start(ent_d[i], ent[:, i * W:(i + 1) * W])


def _get_compiled():
    if "nc" in _CACHE:
        return _CACHE["nc"]
    from contextlib import ExitStack

    import concourse.tile as tile
    from concourse import bacc, mybir

    ops = _register_dve_ops()

    f32 = mybir.dt.float32
    f16 = mybir.dt.float16
    nc = bacc.Bacc("TRN2", target_bir_lowering=False, debug=False)
    x_d = nc.dram_tensor("x_sh", [NIMG, H, W], f32, kind="ExternalInput").ap()
    xt_d = nc.dram_tensor("xt_sh", [NIMG, W, H], f32, kind="ExternalInput").ap()
    sel_d = nc.dram_tensor("sel", [97, NBH * W], f32, kind="ExternalInput").ap()
    band_d = nc.dram_tensor("band16", [H, H], f16, kind="ExternalInput").ap()
    ent_d = nc.dram_tensor("ent", [NIMG, H, W], f32, kind="ExternalOutput").ap()

    with tile.TileContext(nc) as tc:
        with ExitStack() as ctx:
            _emit_kernel(
                nc, tc, ctx, (x_d, xt_d, sel_d, band_d), (ent_d,), ops
            )
    nc.compile()
    _CACHE["nc"] = nc
    return nc


def make_in_maps(x):
    """x: full [8, 3, 96, 96] -> list of 8 per-core input dicts."""
    x = np.ascontiguousarray(np.asarray(x, dtype=np.float32))
    imgs = x.reshape(NCORES * NIMG, H, W)
    sel, band = _build_consts()
    in_maps = []
    for c in range(NCORES):
        sh = np.ascontiguousarray(imgs[c * NIMG:(c + 1) * NIMG])
        in_maps.append(
            {
                "x_sh": sh,
                "xt_sh": np.ascontiguousarray(sh.transpose(0, 2, 1)),
                "sel": sel,
                "band16": band,
            }
        )
    return in_maps


def kernel(x):
    """Full inputs in, full outputs out. x: [8, 3, 96, 96] f32."""
    from concourse.bass_utils import run_bass_kernel_spmd

    nc = _get_compiled()
    in_maps = make_in_maps(x)
    res = run_bass_kernel_spmd(nc, in_maps, list(range(NCORES)))
    out = np.stack([res.results[c]["ent"] for c in range(NCORES)])
    return out.reshape(8, 3, H, W).astype(np.float32)


# revision 13
# speedup vs baseline: 1.8071x; 1.1614x over previous
"""Trainium2 Bass kernel for nn_Entropy (histogram_binning): per-pixel Shannon
entropy of a 5x5-window KDE histogram over 256 intensity bins.

v2 design (Exp-based front end, single activation table, custom DVE ops):
  k(x,b) = sig'(10(x-b)) = u/(1+u)^2 with u = exp(-10|x-b|).
  Layout per (image): h on partitions, free = (bin-block, w-inner) with
  4 zero-pads per 100-col block (plus 5 lead / 2 trail pads per half-chunk).
  Pipeline per superchunk (8 bins x 96 w = 768 cols, 16 sc per half-pair):
    TensorE  D = x - b          (K=97 matmul: stationary [x^T; 1], moving
                                 delta-selector + (-b) row, fp32)
    DVE      a = |D - 128h|     (custom ABSDEV, fp16, strided into padded
                                 chunk; pads preset to 1e4 so exp(pad) = 0)
    ScalarE  u = Exp(-10 a)     (fp16)
    DVE      v = u + u^2(c2+c3u) (custom VPOLY ~ u/(1+u), rel err ~0.5%)
    DVE      A5 = running 5-window of k = v(1-v)  (custom WIN5K scan op:
             state += k(v[i]) - k(v[i-5]); block pads make all SAME edges
             exact with zero fixups)
    TensorE  P = band @ A5      (fp16 H-window band matmul -> PSUM = q)
    ScalarE  L = Ln(q + 2e-6)   (batched; eps-shift identity removes the
                                 per-pixel 1/S scale from the log argument)
    DVE      e = q * L          (PSUM x fp16 -> e-stripe fp16)
  Per half: 7-level pairwise tree-reduce of e over bins -> T = sum_b q ln q.
  Analytic S-path (5 taps of the same fp16 Exp/poly chain on [96,288]) gives
  S = sum_b q; E = ln(S+EPS) - T/(S+EPS).  Exp/Ln/Identity/Abs share one
  activation table -> no ACT table reloads.
  Sharding: B*C = 24 images, 3 per core across 8 cores; no collectives.
"""

import sys

sys.path.insert(0, "/opt/trn_rl_repo")

import numpy as np

H = 96
W = 96
NBH = 128          # bins per half
NIMG = 3
NCORES = 8
EPS = 1e-10
EPS1 = 2e-6
C2P, C3P = -0.89877895, 0.43582129   # v-poly coeffs: v = u + u^2(C2P + C3P u)
BLK = 100          # per-bin block: 4 pads + 96 w
NSC = 16           # superchunks per image (8 bins each, both halves)
BPS = 8            # bins per superchunk per half
HCH = 5 + BPS * BLK + 2   # half-chunk cols: 5 lead + 800 + 2 trail = 807
PAD = 1e4          # a-pad value: exp(-10*PAD) == 0

_CACHE = {}


def _register_dve_ops():
    import concourse.dve_ops as dve_ops
    from concourse.dve_ops import DveOp
    from concourse.dve_spec import (
        C0, C1, AluOp, One, Spec, Src0, Src1, maxx, scan, sq,
    )

    def register(op):
        if op.name not in dve_ops._SUB_OPCODE_FOR_NAME:
            dve_ops.OPS.append(op)
            dve_ops._SUB_OPCODE_FOR_NAME[op.name] = (
                dve_ops._CUSTOM_DVE_ROW_BASE + len(dve_ops.OPS) - 1
            )
        else:
            op = next(o for o in dve_ops.OPS if o.name == op.name)
        return op

    absdev = register(DveOp(
        "ABSDEV_ANT",
        Spec(body=maxx(Src0 - C0, C0 - Src0),
             reference=lambda in0, in1, c0, c1, c2: np.abs(
                 in0.astype(np.float32) - c0).astype(np.float32)),
        subdim=False,
        uops_sha={"v3": "a5866c869c7d6e30", "v4": "006fe4b232e6035a"}))

    vpoly = register(DveOp(
        "VPOLY_ANT",
        Spec(body=Src0 + sq(Src0) * (C0 + C1 * Src0),
             reference=lambda in0, in1, c0, c1, c2: (
                 lambda u: (u + u * u * (c0 + c1 * u)).astype(np.float32)
             )(in0.astype(np.float32))),
        subdim=False,
        perf_en={"v3": True, "v4": True},
        uops_sha={"v3": "217961e937d92645", "v4": "56741f276e7f1259"}))

    win5 = register(DveOp(
        "WIN5K_ANT",
        Spec(body=scan(AluOp.ADD, Src0 * (One - Src0) - Src1 * (One - Src1)),
             reference=lambda in0, in1, c0, c1, c2: np.cumsum(
                 in0.astype(np.float32) * (1 - in0.astype(np.float32))
                 - in1.astype(np.float32) * (1 - in1.astype(np.float32)),
                 axis=-1, dtype=np.float32)),
        subdim=False,
        perf_en={"v3": True, "v4": True},
        uops_sha={"v3": "9d91f28b1ae18abb", "v4": "1425a9f273284709"}))

    return absdev, vpoly, win5


def _patch_act_tables():
    """Force Exp and Ln onto one shared activation table so the table-load
    pass never ping-pongs between per-function tables inside the main loop."""
    import concourse.hw_specs as hw_specs
    from concourse import bacc, mybir

    if getattr(hw_specs, "_ant_act_patch", False):
        return
    AF = mybir.ActivationFunctionType
    orig = hw_specs.get_activation_tables

    def patched(arch):
        tabs = orig(arch)
        out = {}
        for name, s in tabs.items():
            if name == "natural_log_exp_and_others":
                out[name] = set(s)
            else:
                out[name] = set(s) - {AF.Exp, AF.Ln}
        return out

    hw_specs.get_activation_tables = patched
    bacc.get_activation_tables = patched
    hw_specs._ant_act_patch = True


def _build_consts():
    # selector moving operand [97, 128*96]: col c = b_local*96 + w.
    # rows k<96: delta(k == w); row 96: -b_local.
    sel = np.zeros((97, NBH * W), dtype=np.float32)
    cols = np.arange(NBH * W)
    bl = cols // W
    w = cols % W
    sel[w, cols] = 1.0
    sel[96, :] = -bl.astype(np.float32)
    hh = np.arange(H)
    band = (np.abs(hh[:, None] - hh[None, :]) <= 2).astype(np.float16)
    return sel, band


def _emit_kernel(nc, tc, ctx, ins, outs, ops):
    from concourse import mybir

    f32 = mybir.dt.float32
    f16 = mybir.dt.float16
    i32 = mybir.dt.int32
    AF = mybir.ActivationFunctionType
    OP = mybir.AluOpType

    ABSDEV, VPOLY, WIN5 = ops
    x_d, xt_d, sel_d, band_d = ins
    (ent_d,) = outs
    NW = NIMG * W

    consts = ctx.enter_context(tc.tile_pool(name="consts", bufs=1))
    sm = ctx.enter_context(tc.tile_pool(name="sm", bufs=1))
    apool = ctx.enter_context(tc.tile_pool(name="ap", bufs=2))
    upool = ctx.enter_context(tc.tile_pool(name="up", bufs=2))
    vpool = ctx.enter_context(tc.tile_pool(name="vp", bufs=2))
    a5pool = ctx.enter_context(tc.tile_pool(name="a5p", bufs=2))
    lpool = ctx.enter_context(tc.tile_pool(name="lp", bufs=2))
    epool = ctx.enter_context(tc.tile_pool(name="ep", bufs=1))
    tpool = ctx.enter_context(tc.tile_pool(name="tp", bufs=1))
    dpsum = ctx.enter_context(tc.tile_pool(name="dps", bufs=2, space="PSUM"))
    ppsum = ctx.enter_context(tc.tile_pool(name="pps", bufs=1, space="PSUM"))

    # ---- constants / inputs ----
    band_sb = consts.tile([H, H], f16)
    nc.sync.dma_start(band_sb[:], band_d[:])

    xall = consts.tile([H, NW], f32)
    st = consts.tile([97, NIMG * H], f32)
    for i in range(NIMG):
        nc.sync.dma_start(xall[:, i * W:(i + 1) * W], x_d[i])
        nc.sync.dma_start(st[0:96, i * H:(i + 1) * H], xt_d[i])
    nc.vector.memset(st[96:97, :], 1.0)

    # selector DMA'd in 16 chunks so the first D-matmul starts early
    sel_sb = consts.tile([97, NBH * W], f32)
    for j in range(NSC):
        nc.sync.dma_start(
            sel_sb[:, 768 * j:768 * (j + 1)], sel_d[:, 768 * j:768 * (j + 1)])

    bias_tiles = {}

    def bias_ap(val):
        if val not in bias_tiles:
            t = consts.tile([H, 1], f32, tag=f"bias{val}")
            nc.vector.memset(t[:], val)
            bias_tiles[val] = t
        return bias_tiles[val][:]

    # =====================  S path ([96, 288])  =====================
    ni = sm.tile([H, NW], i32)
    nc.vector.tensor_copy(ni[:], xall[:])
    nf = sm.tile([H, NW], f32)
    nc.vector.tensor_copy(nf[:], ni[:])
    ufrac = sm.tile([H, NW], f32)
    nc.vector.tensor_tensor(ufrac[:], xall[:], nf[:], op=OP.subtract)
    taps = (-2, -1, 0, 1, 2)
    atap = sm.tile([H, 5, NW], f16)
    for oi, o in enumerate(taps):
        # a_o = |ufrac - o|; ufrac in [-0.5, 0.5] (i32 copy rounds to nearest)
        nc.vector._custom_dve(
            ABSDEV, out=atap[:, oi, :], in0=ufrac[:], s0=float(o))
    utap = sm.tile([H, 5, NW], f16)
    nc.scalar.activation(utap[:], atap[:], AF.Exp, scale=-10.0)
    vtap = sm.tile([H, 5, NW], f16)
    nc.vector._custom_dve(VPOLY, out=vtap[:], in0=utap[:], s0=C2P, s1=C3P)
    vsq = sm.tile([H, 5, NW], f16)
    nc.vector.tensor_tensor(vsq[:], vtap[:], vtap[:], op=OP.mult)
    ktap = sm.tile([H, 5, NW], f16)
    nc.vector.tensor_tensor(ktap[:], vtap[:], vsq[:], op=OP.subtract)

    spix = sm.tile([H, NW], f32)
    nc.vector.tensor_copy(spix[:], ktap[:, 2, :])  # o=0 tap, always valid
    for oi, o in enumerate(taps):
        if o == 0:
            continue
        m = sm.tile([H, NW], f32, tag=f"m{o}")
        if o < 0:
            nc.vector.tensor_scalar(m[:], nf[:], float(-o), None, op0=OP.is_ge)
        else:
            nc.vector.tensor_scalar(
                m[:], nf[:], float(255 - o), None, op0=OP.is_le)
        tm = sm.tile([H, NW], f32, tag=f"tm{o}")
        nc.vector.tensor_tensor(tm[:], m[:], ktap[:, oi, :], op=OP.mult)
        nc.vector.tensor_tensor(spix[:], spix[:], tm[:], op=OP.add)
    spix16 = sm.tile([H, NW], f16)
    nc.vector.tensor_copy(spix16[:], spix[:])

    ps_s = ppsum.tile([H, 2048], f32, tag="pp")
    nc.tensor.matmul(ps_s[:, 0:NW], band_sb[:], spix16[:], start=True, stop=True)
    sh = sm.tile([H, NW], f32)
    nc.scalar.copy(sh[:], ps_s[:, 0:NW])
    shp = sm.tile([H, NIMG, W + 4], f32)
    nc.vector.memset(shp[:], 0.0)
    for i in range(NIMG):
        nc.vector.tensor_copy(shp[:, i, 2:2 + W], sh[:, i * W:(i + 1) * W])
    swin = sm.tile([H, NIMG, W], f32)
    nc.vector.tensor_tensor(swin[:], shp[:, :, 0:W], shp[:, :, 1:1 + W], op=OP.add)
    for j in (2, 3, 4):
        nc.vector.tensor_tensor(swin[:], swin[:], shp[:, :, j:j + W], op=OP.add)
    sw_flat = swin[:].rearrange("p a b -> p (a b)")
    rtile = sm.tile([H, NW], f32)
    nc.vector.tensor_scalar(rtile[:], sw_flat, EPS, None, op0=OP.add)
    nc.vector.reciprocal(rtile[:], rtile[:])
    lns = sm.tile([H, NW], f32)
    nc.scalar.activation(lns[:], sw_flat, AF.Ln, bias=bias_ap(EPS))

    # =====================  main path  =====================
    QL = sm.tile([H, NW], f32)
    estripes = {}
    for h in range(2):
        est = epool.tile([H, NBH * W], f16, tag=f"e{h}")
        estripes[h] = est
    tspool = tpool

    npads_set = [0]

    for i in range(NIMG):
        for sc in range(NSC):
            dt = dpsum.tile([H, 768], f32, tag="d")
            mvbase = 768 * sc
            nc.tensor.matmul(
                dt[:, 0:512], st[:, i * H:(i + 1) * H],
                sel_sb[:, mvbase:mvbase + 512], start=True, stop=True)
            nc.tensor.matmul(
                dt[:, 512:768], st[:, i * H:(i + 1) * H],
                sel_sb[:, mvbase + 512:mvbase + 768], start=True, stop=True)

            at = apool.tile([H, 2 * HCH], f16, tag="a")
            if npads_set[0] < 2:
                nc.vector.memset(at[:], PAD)
                npads_set[0] += 1
            for h in range(2):
                dst = at[:, h * HCH + 5:h * HCH + 5 + BPS * BLK] \
                    .rearrange("p (b z) -> p b z", z=BLK)[:, :, 4:BLK]
                nc.vector._custom_dve(
                    ABSDEV, out=dst, in0=dt[:, 0:768], s0=128.0 * h)
            ut = upool.tile([H, 2 * HCH], f16, tag="u")
            nc.scalar.activation(ut[:], at[:], AF.Exp, scale=-10.0)
            vt = vpool.tile([H, 2 * HCH], f16, tag="v")
            nc.vector._custom_dve(VPOLY, out=vt[:], in0=ut[:], s0=C2P, s1=C3P)
            a5 = a5pool.tile([H, 2 * HCH], f16, tag="a5")
            nc.vector._custom_dve(
                WIN5, out=a5[:, 0:2 * HCH - 5], in0=vt[:, 5:2 * HCH],
                in1=vt[:, 0:2 * HCH - 5])

            pt = ppsum.tile([H, 2048], f32, tag="pp")
            for h in range(2):
                for pp in range(2):
                    off = h * HCH + 400 * pp + 6
                    mvap = a5[:, off:off + 400] \
                        .rearrange("p (b z) -> p b z", z=BLK)[:, :, 0:96]
                    nc.tensor.matmul(
                        pt[:, 1024 * h + 512 * pp:1024 * h + 512 * pp + 384],
                        band_sb[:], mvap, start=True, stop=True)
            lt = lpool.tile([H, 1536], f16, tag="l")
            nc.scalar.activation(
                lt[:].rearrange("p (a b) -> p a b", b=384),
                pt[:].rearrange("p (a b) -> p a b", b=512)[:, :, 0:384],
                AF.Ln, bias=bias_ap(EPS1))
            for h in range(2):
                nc.vector.tensor_tensor(
                    estripes[h][:, 768 * sc:768 * sc + 768]
                        .rearrange("p (a b) -> p a b", b=384),
                    pt[:, 1024 * h:1024 * h + 1024]
                        .rearrange("p (a b) -> p a b", b=512)[:, :, 0:384],
                    lt[:, 768 * h:768 * h + 768]
                        .rearrange("p (a b) -> p a b", b=384),
                    op=OP.mult)

        # tree reduce per half -> T = sum_b e
        th = {}
        for h in range(2):
            e = estripes[h]
            ts1 = tspool.tile([H, NBH * W // 2], f16, tag=f"ts1_{h}")
            ts2 = tspool.tile([H, NBH * W // 4], f16, tag=f"ts2_{h}")
            nc.vector.tensor_tensor(
                ts1[:, 0:6144], e[:, 0:6144], e[:, 6144:12288], op=OP.add)
            nc.gpsimd.tensor_tensor(
                ts2[:, 0:3072], ts1[:, 0:3072], ts1[:, 3072:6144], op=OP.add)
            nc.gpsimd.tensor_tensor(
                ts1[:, 0:1536], ts2[:, 0:1536], ts2[:, 1536:3072], op=OP.add)
            nc.gpsimd.tensor_tensor(
                ts2[:, 0:768], ts1[:, 0:768], ts1[:, 768:1536], op=OP.add)
            nc.gpsimd.tensor_tensor(
                ts1[:, 0:384], ts2[:, 0:384], ts2[:, 384:768], op=OP.add)
            nc.gpsimd.tensor_tensor(
                ts2[:, 0:192], ts1[:, 0:192], ts1[:, 192:384], op=OP.add)
            tt = sm.tile([H, W], f32, tag=f"th{h}")
            nc.gpsimd.tensor_tensor(
                tt[:], ts2[:, 0:96], ts2[:, 96:192], op=OP.add)
            th[h] = tt
        nc.vector.tensor_tensor(
            QL[:, i * W:(i + 1) * W], th[0][:], th[1][:], op=OP.add)

    # E = lnS - r*T
    ent = sm.tile([H, NW], f32)
    nc.vector.tensor_tensor(ent[:], rtile[:], QL[:], op=OP.mult)
    nc.vector.tensor_tensor(ent[:], lns[:], ent[:], op=OP.subtract)
    for i in range(NIMG):
        nc.sync.dma_start(ent_d[i], ent[:, i * W:(i + 1) * W])


def _get_compiled():
    if "nc" in _CACHE:
        return _CACHE["nc"]
    from contextlib import ExitStack

    import concourse.tile as tile
    from concourse import bacc, mybir

    ops = _register_dve_ops()
    _patch_act_tables()

    f32 = mybir.dt.float32
    f16 = mybir.dt.float16
    nc = bacc.Bacc("TRN2", target_bir_lowering=False, debug=False)
    x_d = nc.dram_tensor("x_sh", [NIMG, H, W], f32, kind="ExternalInput").ap()
    xt_d = nc.dram_tensor("xt_sh", [NIMG, W, H], f32, kind="ExternalInput").ap()
    sel_d = nc.dram_tensor("sel", [97, NBH * W], f32, kind="ExternalInput").ap()
    band_d = nc.dram_tensor("band16", [H, H], f16, kind="ExternalInput").ap()
    ent_d = nc.dram_tensor("ent", [NIMG, H, W], f32, kind="ExternalOutput").ap()

    with tile.TileContext(nc) as tc:
        with ExitStack() as ctx:
            _emit_kernel(
                nc, tc, ctx, (x_d, xt_d, sel_d, band_d), (ent_d,), ops
            )
    nc.compile()
    _CACHE["nc"] = nc
    return nc


def make_in_maps(x):
    """x: full [8, 3, 96, 96] -> list of 8 per-core input dicts."""
    x = np.ascontiguousarray(np.asarray(x, dtype=np.float32))
    imgs = x.reshape(NCORES * NIMG, H, W)
    sel, band = _build_consts()
    in_maps = []
    for c in range(NCORES):
        sh = np.ascontiguousarray(imgs[c * NIMG:(c + 1) * NIMG])
        in_maps.append(
            {
                "x_sh": sh,
                "xt_sh": np.ascontiguousarray(sh.transpose(0, 2, 1)),
                "sel": sel,
                "band16": band,
            }
        )
    return in_maps


def kernel(x):
    """Full inputs in, full outputs out. x: [8, 3, 96, 96] f32."""
    from concourse.bass_utils import run_bass_kernel_spmd

    nc = _get_compiled()
    in_maps = make_in_maps(x)
    res = run_bass_kernel_spmd(nc, in_maps, list(range(NCORES)))
    out = np.stack([res.results[c]["ent"] for c in range(NCORES)])
    return out.reshape(8, 3, H, W).astype(np.float32)


# revision 18
# speedup vs baseline: 1.8192x; 1.0067x over previous
"""Trainium2 Bass kernel for nn_Entropy (histogram_binning): per-pixel Shannon
entropy of a 5x5-window KDE histogram over 256 intensity bins.

v2 design (Exp-based front end, single activation table, custom DVE ops):
  k(x,b) = sig'(10(x-b)) = u/(1+u)^2 with u = exp(-10|x-b|).
  Layout per (image): h on partitions, free = (bin-block, w-inner) with
  4 zero-pads per 100-col block (plus 5 lead / 2 trail pads per half-chunk).
  Pipeline per superchunk (8 bins x 96 w = 768 cols, 16 sc per half-pair):
    TensorE  D = x - b          (K=97 matmul: stationary [x^T; 1], moving
                                 delta-selector + (-b) row, fp32)
    DVE      a = |D - 128h|     (custom ABSDEV, fp16, strided into padded
                                 chunk; pads preset to 1e4 so exp(pad) = 0)
    ScalarE  u = Exp(-10 a)     (fp16)
    DVE      v = u + u^2(c2+c3u) (custom VPOLY ~ u/(1+u), rel err ~0.5%)
    DVE      A5 = running 5-window of k = v(1-v)  (custom WIN5K scan op:
             state += k(v[i]) - k(v[i-5]); block pads make all SAME edges
             exact with zero fixups)
    TensorE  P = band @ A5      (fp16 H-window band matmul -> PSUM = q)
    ScalarE  L = Ln(q + 2e-6)   (batched; eps-shift identity removes the
                                 per-pixel 1/S scale from the log argument)
    DVE      e = q * L          (PSUM x fp16 -> e-stripe fp16)
  Per half: 7-level pairwise tree-reduce of e over bins -> T = sum_b q ln q.
  Analytic S-path (5 taps of the same fp16 Exp/poly chain on [96,288]) gives
  S = sum_b q; E = ln(S+EPS) - T/(S+EPS).  Exp/Ln/Identity/Abs share one
  activation table -> no ACT table reloads.
  Sharding: B*C = 24 images, 3 per core across 8 cores; no collectives.
"""

import sys

sys.path.insert(0, "/opt/trn_rl_repo")

import numpy as np

H = 96
W = 96
NBH = 128          # bins per half
NIMG = 3
NCORES = 8
EPS = 1e-10
EPS1 = 2e-6
C2P, C3P = -0.89877895, 0.43582129   # v-poly coeffs: v = u + u^2(C2P + C3P u)
BLK = 100          # per-bin block: 4 pads + 96 w
NSC = 16           # superchunks per image (8 bins each, both halves)
BPS = 8            # bins per superchunk per half
HCH = 5 + BPS * BLK + 2   # half-chunk cols: 5 lead + 800 + 2 trail = 807
PAD = 1e4          # a-pad value: exp(-10*PAD) == 0

_CACHE = {}


def _register_dve_ops():
    import concourse.dve_ops as dve_ops
    from concourse.dve_ops import DveOp
    from concourse.dve_spec import (
        C0, C1, AluOp, One, Spec, Src0, Src1, maxx, scan, sq,
    )

    def register(op):
        if op.name not in dve_ops._SUB_OPCODE_FOR_NAME:
            dve_ops.OPS.append(op)
            dve_ops._SUB_OPCODE_FOR_NAME[op.name] = (
                dve_ops._CUSTOM_DVE_ROW_BASE + len(dve_ops.OPS) - 1
            )
        else:
            op = next(o for o in dve_ops.OPS if o.name == op.name)
        return op

    absdev = register(DveOp(
        "ABSDEV_ANT",
        Spec(body=maxx(Src0 - C0, C0 - Src0),
             reference=lambda in0, in1, c0, c1, c2: np.abs(
                 in0.astype(np.float32) - c0).astype(np.float32)),
        subdim=False,
        uops_sha={"v3": "a5866c869c7d6e30", "v4": "006fe4b232e6035a"}))

    vpoly = register(DveOp(
        "VPOLY_ANT",
        Spec(body=Src0 + sq(Src0) * (C0 + C1 * Src0),
             reference=lambda in0, in1, c0, c1, c2: (
                 lambda u: (u + u * u * (c0 + c1 * u)).astype(np.float32)
             )(in0.astype(np.float32))),
        subdim=False,
        perf_en={"v3": True, "v4": True},
        uops_sha={"v3": "217961e937d92645", "v4": "56741f276e7f1259"}))

    win5 = register(DveOp(
        "WIN5K_ANT",
        Spec(body=scan(AluOp.ADD, Src0 * (One - Src0) - Src1 * (One - Src1)),
             reference=lambda in0, in1, c0, c1, c2: np.cumsum(
                 in0.astype(np.float32) * (1 - in0.astype(np.float32))
                 - in1.astype(np.float32) * (1 - in1.astype(np.float32)),
                 axis=-1, dtype=np.float32)),
        subdim=False,
        perf_en={"v3": True, "v4": True},
        uops_sha={"v3": "9d91f28b1ae18abb", "v4": "1425a9f273284709"}))

    return absdev, vpoly, win5


def _patch_act_tables():
    """Force Exp and Ln onto one shared activation table so the table-load
    pass never ping-pongs between per-function tables inside the main loop."""
    import concourse.hw_specs as hw_specs
    from concourse import bacc, mybir

    if getattr(hw_specs, "_ant_act_patch", False):
        return
    AF = mybir.ActivationFunctionType
    orig = hw_specs.get_activation_tables

    def patched(arch):
        tabs = orig(arch)
        out = {}
        for name, s in tabs.items():
            if name == "natural_log_exp_and_others":
                out[name] = set(s)
            else:
                out[name] = set(s) - {AF.Exp, AF.Ln}
        return out

    hw_specs.get_activation_tables = patched
    bacc.get_activation_tables = patched
    hw_specs._ant_act_patch = True


def _build_consts():
    # selector moving operand [97, 128*96]: col c = b_local*96 + w.
    # rows k<96: delta(k == w); row 96: -b_local.
    sel = np.zeros((97, NBH * W), dtype=np.float32)
    cols = np.arange(NBH * W)
    bl = cols // W
    w = cols % W
    sel[w, cols] = 1.0
    sel[96, :] = -bl.astype(np.float32)
    hh = np.arange(H)
    band = (np.abs(hh[:, None] - hh[None, :]) <= 2).astype(np.float16)
    return sel, band


def _emit_kernel(nc, tc, ctx, ins, outs, ops):
    from concourse import mybir

    f32 = mybir.dt.float32
    f16 = mybir.dt.float16
    i32 = mybir.dt.int32
    AF = mybir.ActivationFunctionType
    OP = mybir.AluOpType

    ABSDEV, VPOLY, WIN5 = ops
    x_d, xt_d, sel_d, band_d = ins
    (ent_d,) = outs
    NW = NIMG * W

    consts = ctx.enter_context(tc.tile_pool(name="consts", bufs=1))
    sm = ctx.enter_context(tc.tile_pool(name="sm", bufs=1))
    apool = ctx.enter_context(tc.tile_pool(name="ap", bufs=2))
    upool = ctx.enter_context(tc.tile_pool(name="up", bufs=2))
    vpool = ctx.enter_context(tc.tile_pool(name="vp", bufs=2))
    a5pool = ctx.enter_context(tc.tile_pool(name="a5p", bufs=2))
    lpool = ctx.enter_context(tc.tile_pool(name="lp", bufs=2))
    epool = ctx.enter_context(tc.tile_pool(name="ep", bufs=1))
    tpool = ctx.enter_context(tc.tile_pool(name="tp", bufs=1))
    dpsum = ctx.enter_context(tc.tile_pool(name="dps", bufs=2, space="PSUM"))
    ppsum = ctx.enter_context(tc.tile_pool(name="pps", bufs=1, space="PSUM"))

    # ---- constants / inputs ----
    band_sb = consts.tile([H, H], f16)
    nc.sync.dma_start(band_sb[:], band_d[:])

    xall = consts.tile([H, NW], f32)
    st = consts.tile([97, NIMG * H], f32)
    for i in range(NIMG):
        nc.sync.dma_start(xall[:, i * W:(i + 1) * W], x_d[i])
        nc.sync.dma_start(st[0:96, i * H:(i + 1) * H], xt_d[i])
    nc.vector.memset(st[96:97, :], 1.0)

    # selector DMA'd in 16 chunks, spread over 4 queues, so the first
    # D-matmul starts early and the transfers run in parallel
    sel_sb = consts.tile([97, NBH * W], f32)
    qengines = (nc.sync, nc.gpsimd, nc.scalar)
    for j in range(NSC):
        qengines[j % 3].dma_start(
            sel_sb[:, 768 * j:768 * (j + 1)], sel_d[:, 768 * j:768 * (j + 1)])

    bias_tiles = {}

    def bias_ap(val):
        if val not in bias_tiles:
            t = consts.tile([H, 1], f32, tag=f"bias{val}")
            nc.vector.memset(t[:], val)
            bias_tiles[val] = t
        return bias_tiles[val][:]

    # =====================  S path ([96, 288])  =====================
    ni = sm.tile([H, NW], i32)
    nc.vector.tensor_copy(ni[:], xall[:])
    nf = sm.tile([H, NW], f32)
    nc.vector.tensor_copy(nf[:], ni[:])
    ufrac = sm.tile([H, NW], f32)
    nc.vector.tensor_tensor(ufrac[:], xall[:], nf[:], op=OP.subtract)
    taps = (-2, -1, 0, 1, 2)
    atap = sm.tile([H, 5, NW], f16)
    for oi, o in enumerate(taps):
        # a_o = |ufrac - o|; ufrac in [-0.5, 0.5] (i32 copy rounds to nearest)
        nc.vector._custom_dve(
            ABSDEV, out=atap[:, oi, :], in0=ufrac[:], s0=float(o))
    utap = sm.tile([H, 5, NW], f16)
    nc.scalar.activation(utap[:], atap[:], AF.Exp, scale=-10.0)
    vtap = sm.tile([H, 5, NW], f16)
    nc.vector._custom_dve(VPOLY, out=vtap[:], in0=utap[:], s0=C2P, s1=C3P)
    vsq = sm.tile([H, 5, NW], f16)
    nc.vector.tensor_tensor(vsq[:], vtap[:], vtap[:], op=OP.mult)
    ktap = sm.tile([H, 5, NW], f16)
    nc.vector.tensor_tensor(ktap[:], vtap[:], vsq[:], op=OP.subtract)

    spix = sm.tile([H, NW], f32)
    nc.vector.tensor_copy(spix[:], ktap[:, 2, :])  # o=0 tap, always valid
    for oi, o in enumerate(taps):
        if o == 0:
            continue
        m = sm.tile([H, NW], f32, tag=f"m{o}")
        if o < 0:
            nc.vector.tensor_scalar(m[:], nf[:], float(-o), None, op0=OP.is_ge)
        else:
            nc.vector.tensor_scalar(
                m[:], nf[:], float(255 - o), None, op0=OP.is_le)
        tm = sm.tile([H, NW], f32, tag=f"tm{o}")
        nc.vector.tensor_tensor(tm[:], m[:], ktap[:, oi, :], op=OP.mult)
        nc.vector.tensor_tensor(spix[:], spix[:], tm[:], op=OP.add)
    spix16 = sm.tile([H, NW], f16)
    nc.vector.tensor_copy(spix16[:], spix[:])

    ps_s = ppsum.tile([H, 2048], f32, tag="pp")
    nc.tensor.matmul(ps_s[:, 0:NW], band_sb[:], spix16[:], start=True, stop=True)
    sh = sm.tile([H, NW], f32)
    nc.scalar.copy(sh[:], ps_s[:, 0:NW])
    shp = sm.tile([H, NIMG, W + 4], f32)
    nc.vector.memset(shp[:], 0.0)
    for i in range(NIMG):
        nc.vector.tensor_copy(shp[:, i, 2:2 + W], sh[:, i * W:(i + 1) * W])
    swin = sm.tile([H, NIMG, W], f32)
    nc.vector.tensor_tensor(swin[:], shp[:, :, 0:W], shp[:, :, 1:1 + W], op=OP.add)
    for j in (2, 3, 4):
        nc.vector.tensor_tensor(swin[:], swin[:], shp[:, :, j:j + W], op=OP.add)
    sw_flat = swin[:].rearrange("p a b -> p (a b)")
    rtile = sm.tile([H, NW], f32)
    nc.vector.tensor_scalar(rtile[:], sw_flat, EPS, None, op0=OP.add)
    nc.vector.reciprocal(rtile[:], rtile[:])
    lns = sm.tile([H, NW], f32)
    nc.scalar.activation(lns[:], sw_flat, AF.Ln, bias=bias_ap(EPS))

    # =====================  main path  =====================
    QL = sm.tile([H, NW], f32)
    estripes = {}
    for h in range(2):
        est = epool.tile([H, NBH * W], f16, tag=f"e{h}")
        estripes[h] = est
    tspool = tpool

    npads_set = [0]

    for i in range(NIMG):
        for sc in range(NSC):
            dt = dpsum.tile([H, 768], f32, tag="d")
            mvbase = 768 * sc
            nc.tensor.matmul(
                dt[:, 0:512], st[:, i * H:(i + 1) * H],
                sel_sb[:, mvbase:mvbase + 512], start=True, stop=True)
            nc.tensor.matmul(
                dt[:, 512:768], st[:, i * H:(i + 1) * H],
                sel_sb[:, mvbase + 512:mvbase + 768], start=True, stop=True)

            at = apool.tile([H, 2 * HCH], f16, tag="a")
            if npads_set[0] < 2:
                nc.vector.memset(at[:], PAD)
                npads_set[0] += 1
            dst0 = at[:, 5:5 + BPS * BLK] \
                .rearrange("p (b z) -> p b z", z=BLK)[:, :, 4:BLK]
            nc.vector._custom_dve(ABSDEV, out=dst0, in0=dt[:, 0:768], s0=0.0)
            dst1 = at[:, HCH + 5:HCH + 5 + BPS * BLK] \
                .rearrange("p (b z) -> p b z", z=BLK)[:, :, 4:BLK]
            nc.scalar.activation(dst1, dt[:, 0:768], AF.Abs, bias=bias_ap(-128.0))
            ut = upool.tile([H, 2 * HCH], f16, tag="u")
            nc.scalar.activation(ut[:], at[:], AF.Exp, scale=-10.0)
            vt = vpool.tile([H, 2 * HCH], f16, tag="v")
            nc.vector._custom_dve(VPOLY, out=vt[:], in0=ut[:], s0=C2P, s1=C3P)
            a5 = a5pool.tile([H, 2 * HCH], f16, tag="a5")
            nc.vector._custom_dve(
                WIN5, out=a5[:, 0:2 * HCH - 5], in0=vt[:, 5:2 * HCH],
                in1=vt[:, 0:2 * HCH - 5])

            pt = ppsum.tile([H, 2048], f32, tag="pp")
            for h in range(2):
                for pp in range(2):
                    off = h * HCH + 400 * pp + 6
                    mvap = a5[:, off:off + 400] \
                        .rearrange("p (b z) -> p b z", z=BLK)[:, :, 0:96]
                    nc.tensor.matmul(
                        pt[:, 1024 * h + 512 * pp:1024 * h + 512 * pp + 384],
                        band_sb[:], mvap, start=True, stop=True)
            lt = lpool.tile([H, 1536], f16, tag="l")
            nc.scalar.activation(
                lt[:].rearrange("p (a b) -> p a b", b=384),
                pt[:].rearrange("p (a b) -> p a b", b=512)[:, :, 0:384],
                AF.Ln, bias=bias_ap(EPS1))
            for h in range(2):
                nc.vector.tensor_tensor(
                    estripes[h][:, 768 * sc:768 * sc + 768]
                        .rearrange("p (a b) -> p a b", b=384),
                    pt[:, 1024 * h:1024 * h + 1024]
                        .rearrange("p (a b) -> p a b", b=512)[:, :, 0:384],
                    lt[:, 768 * h:768 * h + 768]
                        .rearrange("p (a b) -> p a b", b=384),
                    op=OP.mult)

        # tree reduce per half -> T = sum_b e
        th = {}
        for h in range(2):
            e = estripes[h]
            ts1 = tspool.tile([H, NBH * W // 2], f16, tag=f"ts1_{h}")
            ts2 = tspool.tile([H, NBH * W // 4], f16, tag=f"ts2_{h}")
            nc.gpsimd.tensor_tensor(
                ts1[:, 0:6144], e[:, 0:6144], e[:, 6144:12288], op=OP.add)
            nc.gpsimd.tensor_tensor(
                ts2[:, 0:3072], ts1[:, 0:3072], ts1[:, 3072:6144], op=OP.add)
            nc.gpsimd.tensor_tensor(
                ts1[:, 0:1536], ts2[:, 0:1536], ts2[:, 1536:3072], op=OP.add)
            nc.gpsimd.tensor_tensor(
                ts2[:, 0:768], ts1[:, 0:768], ts1[:, 768:1536], op=OP.add)
            nc.gpsimd.tensor_tensor(
                ts1[:, 0:384], ts2[:, 0:384], ts2[:, 384:768], op=OP.add)
            nc.gpsimd.tensor_tensor(
                ts2[:, 0:192], ts1[:, 0:192], ts1[:, 192:384], op=OP.add)
            tt = sm.tile([H, W], f32, tag=f"th{h}")
            nc.gpsimd.tensor_tensor(
                tt[:], ts2[:, 0:96], ts2[:, 96:192], op=OP.add)
            th[h] = tt
        nc.vector.tensor_tensor(
            QL[:, i * W:(i + 1) * W], th[0][:], th[1][:], op=OP.add)

    # E = lnS - r*T
    ent = sm.tile([H, NW], f32)
    nc.vector.tensor_tensor(ent[:], rtile[:], QL[:], op=OP.mult)
    nc.vector.tensor_tensor(ent[:], lns[:], ent[:], op=OP.subtract)
    for i in range(NIMG):
        nc.sync.dma_start(ent_d[i], ent[:, i * W:(i + 1) * W])


def _get_compiled():
    if "nc" in _CACHE:
        return _CACHE["nc"]
    from contextlib import ExitStack

    import concourse.tile as tile
    from concourse import bacc, mybir

    ops = _register_dve_ops()
    _patch_act_tables()

    f32 = mybir.dt.float32
    f16 = mybir.dt.float16
    nc = bacc.Bacc("TRN2", target_bir_lowering=False, debug=False)
    x_d = nc.dram_tensor("x_sh", [NIMG, H, W], f32, kind="ExternalInput").ap()
    xt_d = nc.dram_tensor("xt_sh", [NIMG, W, H], f32, kind="ExternalInput").ap()
    sel_d = nc.dram_tensor("sel", [97, NBH * W], f32, kind="ExternalInput").ap()
    band_d = nc.dram_tensor("band16", [H, H], f16, kind="ExternalInput").ap()
    ent_d = nc.dram_tensor("ent", [NIMG, H, W], f32, kind="ExternalOutput").ap()

    with tile.TileContext(nc) as tc:
        with ExitStack() as ctx:
            _emit_kernel(
                nc, tc, ctx, (x_d, xt_d, sel_d, band_d), (ent_d,), ops
            )
    nc.compile()
    _CACHE["nc"] = nc
    return nc


def make_in_maps(x):
    """x: full [8, 3, 96, 96] -> list of 8 per-core input dicts."""
    x = np.ascontiguousarray(np.asarray(x, dtype=np.float32))
    imgs = x.reshape(NCORES * NIMG, H, W)
    sel, band = _build_consts()
    in_maps = []
    for c in range(NCORES):
        sh = np.ascontiguousarray(imgs[c * NIMG:(c + 1) * NIMG])
        in_maps.append(
            {
                "x_sh": sh,
                "xt_sh": np.ascontiguousarray(sh.transpose(0, 2, 1)),
                "sel": sel,
                "band16": band,
            }
        )
    return in_maps


def kernel(x):
    """Full inputs in, full outputs out. x: [8, 3, 96, 96] f32."""
    from concourse.bass_utils import run_bass_kernel_spmd

    nc = _get_compiled()
    in_maps = make_in_maps(x)
    res = run_bass_kernel_spmd(nc, in_maps, list(range(NCORES)))
    out = np.stack([res.results[c]["ent"] for c in range(NCORES)])
    return out.reshape(8, 3, H, W).astype(np.float32)


# revision 23
# speedup vs baseline: 2.1941x; 1.2061x over previous
"""Trainium2 Bass kernel for nn_Entropy (histogram_binning): per-pixel Shannon
entropy of a 5x5-window KDE histogram over 256 intensity bins.

v2 design (Exp-based front end, single activation table, custom DVE ops):
  k(x,b) = sig'(10(x-b)) = u/(1+u)^2 with u = exp(-10|x-b|).
  Layout per (image): h on partitions, free = (bin-block, w-inner) with
  4 zero-pads per 100-col block (plus 5 lead / 2 trail pads per half-chunk).
  Pipeline per superchunk (8 bins x 96 w = 768 cols, 16 sc per half-pair):
    TensorE  D = x - b          (K=97 matmul: stationary [x^T; 1], moving
                                 delta-selector + (-b) row, fp32)
    DVE      a = |D - 128h|     (custom ABSDEV, fp16, strided into padded
                                 chunk; pads preset to 1e4 so exp(pad) = 0)
    ScalarE  u = Exp(-10 a)     (fp16)
    DVE      v = u + u^2(c2+c3u) (custom VPOLY ~ u/(1+u), rel err ~0.5%)
    DVE      A5 = running 5-window of k = v(1-v)  (custom WIN5K scan op:
             state += k(v[i]) - k(v[i-5]); block pads make all SAME edges
             exact with zero fixups)
    TensorE  P = band @ A5      (fp16 H-window band matmul -> PSUM = q)
    ScalarE  L = Ln(q + 2e-6)   (batched; eps-shift identity removes the
                                 per-pixel 1/S scale from the log argument)
    DVE      e = q * L          (PSUM x fp16 -> e-stripe fp16)
  Per half: 7-level pairwise tree-reduce of e over bins -> T = sum_b q ln q.
  Analytic S-path (5 taps of the same fp16 Exp/poly chain on [96,288]) gives
  S = sum_b q; E = ln(S+EPS) - T/(S+EPS).  Exp/Ln/Identity/Abs share one
  activation table -> no ACT table reloads.
  Sharding: B*C = 24 images, 3 per core across 8 cores; no collectives.
"""

import sys

sys.path.insert(0, "/opt/trn_rl_repo")

import numpy as np

H = 96
W = 96
NBH = 128          # bins per half
NIMG = 3
NCORES = 8
EPS = 1e-10
EPS1 = 2e-6
C2P, C3P = -0.89877895, 0.43582129   # v-poly coeffs: v = u + u^2(C2P + C3P u)
BLK = 100          # per-bin block: 4 pads + 96 w
NSC = 16           # superchunks per image (8 bins each, both halves)
BPS = 8            # bins per superchunk per half
HCH = 5 + BPS * BLK + 2   # half-chunk cols: 5 lead + 800 + 2 trail = 807
PAD = 1e4          # a-pad value: exp(-10*PAD) == 0

_CACHE = {}


def _register_dve_ops():
    import concourse.dve_ops as dve_ops
    from concourse.dve_ops import DveOp
    from concourse.dve_spec import (
        C0, C1, AluOp, One, Spec, Src0, Src1, maxx, scan, sq,
    )

    def register(op):
        if op.name not in dve_ops._SUB_OPCODE_FOR_NAME:
            dve_ops.OPS.append(op)
            dve_ops._SUB_OPCODE_FOR_NAME[op.name] = (
                dve_ops._CUSTOM_DVE_ROW_BASE + len(dve_ops.OPS) - 1
            )
        else:
            op = next(o for o in dve_ops.OPS if o.name == op.name)
        return op

    absdev = register(DveOp(
        "ABSDEV_ANT",
        Spec(body=maxx(Src0 - C0, C0 - Src0),
             reference=lambda in0, in1, c0, c1, c2: np.abs(
                 in0.astype(np.float32) - c0).astype(np.float32)),
        subdim=False,
        uops_sha={"v3": "a5866c869c7d6e30", "v4": "006fe4b232e6035a"}))

    vpoly = register(DveOp(
        "VPOLY_ANT",
        Spec(body=Src0 + sq(Src0) * (C0 + C1 * Src0),
             reference=lambda in0, in1, c0, c1, c2: (
                 lambda u: (u + u * u * (c0 + c1 * u)).astype(np.float32)
             )(in0.astype(np.float32))),
        subdim=False,
        perf_en={"v3": True, "v4": True},
        uops_sha={"v3": "217961e937d92645", "v4": "56741f276e7f1259"}))

    win5 = register(DveOp(
        "WIN5K_ANT",
        Spec(body=scan(AluOp.ADD, Src0 * (One - Src0) - Src1 * (One - Src1)),
             reference=lambda in0, in1, c0, c1, c2: np.cumsum(
                 in0.astype(np.float32) * (1 - in0.astype(np.float32))
                 - in1.astype(np.float32) * (1 - in1.astype(np.float32)),
                 axis=-1, dtype=np.float32)),
        subdim=False,
        perf_en={"v3": True, "v4": True},
        uops_sha={"v3": "9d91f28b1ae18abb", "v4": "1425a9f273284709"}))

    return absdev, vpoly, win5


def _patch_act_tables():
    """Force Exp and Ln onto one shared activation table so the table-load
    pass never ping-pongs between per-function tables inside the main loop."""
    import concourse.hw_specs as hw_specs
    from concourse import bacc, mybir

    if getattr(hw_specs, "_ant_act_patch", False):
        return
    AF = mybir.ActivationFunctionType
    orig = hw_specs.get_activation_tables

    def patched(arch):
        tabs = orig(arch)
        out = {}
        for name, s in tabs.items():
            if name == "natural_log_exp_and_others":
                out[name] = set(s)
            else:
                out[name] = set(s) - {AF.Exp, AF.Ln}
        return out

    hw_specs.get_activation_tables = patched
    bacc.get_activation_tables = patched
    hw_specs._ant_act_patch = True


def _build_consts():
    # selector moving operand [97, 128*96]: col c = b_local*96 + w.
    # rows k<96: delta(k == w); row 96: -b_local.
    sel = np.zeros((97, NBH * W), dtype=np.float32)
    cols = np.arange(NBH * W)
    bl = cols // W
    w = cols % W
    sel[w, cols] = 1.0
    sel[96, :] = -bl.astype(np.float32)
    hh = np.arange(H)
    band = (np.abs(hh[:, None] - hh[None, :]) <= 2).astype(np.float16)
    return sel, band


def _emit_kernel(nc, tc, ctx, ins, outs, ops):
    from concourse import mybir

    f32 = mybir.dt.float32
    f16 = mybir.dt.float16
    i32 = mybir.dt.int32
    AF = mybir.ActivationFunctionType
    OP = mybir.AluOpType

    ABSDEV, VPOLY, WIN5 = ops
    x_d, xt_d, sel_d, band_d = ins
    (ent_d,) = outs
    NW = NIMG * W

    consts = ctx.enter_context(tc.tile_pool(name="consts", bufs=1))
    sm = ctx.enter_context(tc.tile_pool(name="sm", bufs=1))
    apool = ctx.enter_context(tc.tile_pool(name="ap", bufs=2))
    upool = ctx.enter_context(tc.tile_pool(name="up", bufs=2))
    vpool = ctx.enter_context(tc.tile_pool(name="vp", bufs=2))
    a5pool = ctx.enter_context(tc.tile_pool(name="a5p", bufs=2))
    lpool = ctx.enter_context(tc.tile_pool(name="lp", bufs=2))
    epool = ctx.enter_context(tc.tile_pool(name="ep", bufs=1))
    tpool = ctx.enter_context(tc.tile_pool(name="tp", bufs=1))
    dpsum = ctx.enter_context(tc.tile_pool(name="dps", bufs=2, space="PSUM"))
    ppsum = ctx.enter_context(tc.tile_pool(name="pps", bufs=1, space="PSUM"))

    # ---- constants / inputs ----
    band_sb = consts.tile([H, H], f16)
    nc.sync.dma_start(band_sb[:], band_d[:])

    xall = consts.tile([H, NW], f32)
    st = consts.tile([97, NIMG * H], f32)
    for i in range(NIMG):
        nc.sync.dma_start(xall[:, i * W:(i + 1) * W], x_d[i])
        nc.sync.dma_start(st[0:96, i * H:(i + 1) * H], xt_d[i])
    nc.vector.memset(st[96:97, :], 1.0)

    # selector DMA'd in 16 chunks, spread over 4 queues, so the first
    # D-matmul starts early and the transfers run in parallel
    sel_sb = consts.tile([97, NBH * W], f32)
    qengines = (nc.sync, nc.gpsimd, nc.scalar)
    for j in range(NSC):
        qengines[j % 3].dma_start(
            sel_sb[:, 768 * j:768 * (j + 1)], sel_d[:, 768 * j:768 * (j + 1)])

    bias_tiles = {}

    def bias_ap(val):
        if val not in bias_tiles:
            t = consts.tile([H, 1], f32, tag=f"bias{val}")
            nc.vector.memset(t[:], val)
            bias_tiles[val] = t
        return bias_tiles[val][:]

    # =====================  S path ([96, 288])  =====================
    ni = sm.tile([H, NW], i32)
    nc.vector.tensor_copy(ni[:], xall[:])
    nf = sm.tile([H, NW], f32)
    nc.vector.tensor_copy(nf[:], ni[:])
    ufrac = sm.tile([H, NW], f32)
    nc.vector.tensor_tensor(ufrac[:], xall[:], nf[:], op=OP.subtract)
    taps = (-2, -1, 0, 1, 2)
    atap = sm.tile([H, 5, NW], f16)
    for oi, o in enumerate(taps):
        # a_o = |ufrac - o|; ufrac in [-0.5, 0.5] (i32 copy rounds to nearest)
        nc.vector._custom_dve(
            ABSDEV, out=atap[:, oi, :], in0=ufrac[:], s0=float(o))
    utap = sm.tile([H, 5, NW], f16)
    nc.scalar.activation(utap[:], atap[:], AF.Exp, scale=-10.0)
    vtap = sm.tile([H, 5, NW], f16)
    nc.vector._custom_dve(VPOLY, out=vtap[:], in0=utap[:], s0=C2P, s1=C3P)
    vsq = sm.tile([H, 5, NW], f16)
    nc.vector.tensor_tensor(vsq[:], vtap[:], vtap[:], op=OP.mult)
    ktap = sm.tile([H, 5, NW], f16)
    nc.vector.tensor_tensor(ktap[:], vtap[:], vsq[:], op=OP.subtract)

    spix = sm.tile([H, NW], f32)
    nc.vector.tensor_copy(spix[:], ktap[:, 2, :])  # o=0 tap, always valid
    for oi, o in enumerate(taps):
        if o == 0:
            continue
        m = sm.tile([H, NW], f32, tag=f"m{o}")
        if o < 0:
            nc.vector.tensor_scalar(m[:], nf[:], float(-o), None, op0=OP.is_ge)
        else:
            nc.vector.tensor_scalar(
                m[:], nf[:], float(255 - o), None, op0=OP.is_le)
        tm = sm.tile([H, NW], f32, tag=f"tm{o}")
        nc.vector.tensor_tensor(tm[:], m[:], ktap[:, oi, :], op=OP.mult)
        nc.vector.tensor_tensor(spix[:], spix[:], tm[:], op=OP.add)
    spix16 = sm.tile([H, NW], f16)
    nc.vector.tensor_copy(spix16[:], spix[:])

    ps_s = ppsum.tile([H, 2048], f32, tag="pp")
    nc.tensor.matmul(ps_s[:, 0:NW], band_sb[:], spix16[:], start=True, stop=True)
    sh = sm.tile([H, NW], f32)
    nc.scalar.copy(sh[:], ps_s[:, 0:NW])
    shp = sm.tile([H, NIMG, W + 4], f32)
    nc.vector.memset(shp[:], 0.0)
    for i in range(NIMG):
        nc.vector.tensor_copy(shp[:, i, 2:2 + W], sh[:, i * W:(i + 1) * W])
    swin = sm.tile([H, NIMG, W], f32)
    nc.vector.tensor_tensor(swin[:], shp[:, :, 0:W], shp[:, :, 1:1 + W], op=OP.add)
    for j in (2, 3, 4):
        nc.vector.tensor_tensor(swin[:], swin[:], shp[:, :, j:j + W], op=OP.add)
    sw_flat = swin[:].rearrange("p a b -> p (a b)")
    rtile = sm.tile([H, NW], f32)
    nc.vector.tensor_scalar(rtile[:], sw_flat, EPS, None, op0=OP.add)
    nc.vector.reciprocal(rtile[:], rtile[:])
    lns = sm.tile([H, NW], f32)
    nc.scalar.activation(lns[:], sw_flat, AF.Ln, bias=bias_ap(EPS))

    # =====================  main path  =====================
    QL = sm.tile([H, NW], f32)
    taccs = {}
    for h in range(2):
        tacc = sm.tile([H, W], f32, tag=f"tacc{h}")
        taccs[h] = tacc

    npads_set = [0]

    for i in range(NIMG):
        for h in range(2):
            nc.vector.memset(taccs[h][:], 0.0)
        for sc in range(NSC):
            dt = dpsum.tile([H, 768], f32, tag="d")
            mvbase = 768 * sc
            nc.tensor.matmul(
                dt[:, 0:512], st[:, i * H:(i + 1) * H],
                sel_sb[:, mvbase:mvbase + 512], start=True, stop=True)
            nc.tensor.matmul(
                dt[:, 512:768], st[:, i * H:(i + 1) * H],
                sel_sb[:, mvbase + 512:mvbase + 768], start=True, stop=True)

            at = apool.tile([H, 2 * HCH], f16, tag="a")
            if npads_set[0] < 2:
                nc.vector.memset(at[:], PAD)
                npads_set[0] += 1
            dst0 = at[:, 5:5 + BPS * BLK] \
                .rearrange("p (b z) -> p b z", z=BLK)[:, :, 4:BLK]
            nc.vector._custom_dve(ABSDEV, out=dst0, in0=dt[:, 0:768], s0=0.0)
            dst1 = at[:, HCH + 5:HCH + 5 + BPS * BLK] \
                .rearrange("p (b z) -> p b z", z=BLK)[:, :, 4:BLK]
            nc.scalar.activation(dst1, dt[:, 0:768], AF.Abs, bias=bias_ap(-128.0))
            ut = upool.tile([H, 2 * HCH], f16, tag="u")
            nc.scalar.activation(ut[:], at[:], AF.Exp, scale=-10.0)
            vt = vpool.tile([H, 2 * HCH], f16, tag="v")
            nc.vector._custom_dve(VPOLY, out=vt[:], in0=ut[:], s0=C2P, s1=C3P)
            a5 = a5pool.tile([H, 2 * HCH], f16, tag="a5")
            nc.vector._custom_dve(
                WIN5, out=a5[:, 0:2 * HCH - 5], in0=vt[:, 5:2 * HCH],
                in1=vt[:, 0:2 * HCH - 5])

            pt = ppsum.tile([H, 2048], f32, tag="pp")
            for h in range(2):
                for pp in range(2):
                    off = h * HCH + 400 * pp + 6
                    mvap = a5[:, off:off + 400] \
                        .rearrange("p (b z) -> p b z", z=BLK)[:, :, 0:96]
                    nc.tensor.matmul(
                        pt[:, 1024 * h + 512 * pp:1024 * h + 512 * pp + 384],
                        band_sb[:], mvap, start=True, stop=True)
            lt = lpool.tile([H, 1536], f16, tag="l")
            nc.scalar.activation(
                lt[:].rearrange("p (a b) -> p a b", b=384),
                pt[:].rearrange("p (a b) -> p a b", b=512)[:, :, 0:384],
                AF.Ln, bias=bias_ap(EPS1))
            for h in range(2):
                et = epool.tile([H, 768 + 384], f16, tag=f"e{h}")
                nc.vector.tensor_tensor(
                    et[:, 0:768].rearrange("p (a b) -> p a b", b=384),
                    pt[:, 1024 * h:1024 * h + 1024]
                        .rearrange("p (a b) -> p a b", b=512)[:, :, 0:384],
                    lt[:, 768 * h:768 * h + 768]
                        .rearrange("p (a b) -> p a b", b=384),
                    op=OP.mult)
                # per-sc tree over the 8 bins -> accumulate into tacc
                nc.gpsimd.tensor_tensor(
                    et[:, 768:1152], et[:, 0:384], et[:, 384:768], op=OP.add)
                nc.gpsimd.tensor_tensor(
                    et[:, 0:192], et[:, 768:960], et[:, 960:1152], op=OP.add)
                nc.gpsimd.tensor_tensor(
                    et[:, 192:288], et[:, 0:96], et[:, 96:192], op=OP.add)
                nc.gpsimd.tensor_tensor(
                    taccs[h][:], taccs[h][:], et[:, 192:288], op=OP.add)

        nc.vector.tensor_tensor(
            QL[:, i * W:(i + 1) * W], taccs[0][:], taccs[1][:], op=OP.add)

    # E = lnS - r*T
    ent = sm.tile([H, NW], f32)
    nc.vector.tensor_tensor(ent[:], rtile[:], QL[:], op=OP.mult)
    nc.vector.tensor_tensor(ent[:], lns[:], ent[:], op=OP.subtract)
    for i in range(NIMG):
        nc.sync.dma_start(ent_d[i], ent[:, i * W:(i + 1) * W])


def _get_compiled():
    if "nc" in _CACHE:
        return _CACHE["nc"]
    from contextlib import ExitStack

    import concourse.tile as tile
    from concourse import bacc, mybir

    ops = _register_dve_ops()
    _patch_act_tables()

    f32 = mybir.dt.float32
    f16 = mybir.dt.float16
    nc = bacc.Bacc("TRN2", target_bir_lowering=False, debug=False)
    x_d = nc.dram_tensor("x_sh", [NIMG, H, W], f32, kind="ExternalInput").ap()
    xt_d = nc.dram_tensor("xt_sh", [NIMG, W, H], f32, kind="ExternalInput").ap()
    sel_d = nc.dram_tensor("sel", [97, NBH * W], f32, kind="ExternalInput").ap()
    band_d = nc.dram_tensor("band16", [H, H], f16, kind="ExternalInput").ap()
    ent_d = nc.dram_tensor("ent", [NIMG, H, W], f32, kind="ExternalOutput").ap()

    with tile.TileContext(nc) as tc:
        with ExitStack() as ctx:
            _emit_kernel(
                nc, tc, ctx, (x_d, xt_d, sel_d, band_d), (ent_d,), ops
            )
    nc.compile()
    _CACHE["nc"] = nc
    return nc


def make_in_maps(x):
    """x: full [8, 3, 96, 96] -> list of 8 per-core input dicts."""
    x = np.ascontiguousarray(np.asarray(x, dtype=np.float32))
    imgs = x.reshape(NCORES * NIMG, H, W)
    sel, band = _build_consts()
    in_maps = []
    for c in range(NCORES):
        sh = np.ascontiguousarray(imgs[c * NIMG:(c + 1) * NIMG])
        in_maps.append(
            {
                "x_sh": sh,
                "xt_sh": np.ascontiguousarray(sh.transpose(0, 2, 1)),
                "sel": sel,
                "band16": band,
            }
        )
    return in_maps


def kernel(x):
    """Full inputs in, full outputs out. x: [8, 3, 96, 96] f32."""
    from concourse.bass_utils import run_bass_kernel_spmd

    nc = _get_compiled()
    in_maps = make_in_maps(x)
    res = run_bass_kernel_spmd(nc, in_maps, list(range(NCORES)))
    out = np.stack([res.results[c]["ent"] for c in range(NCORES)])
    return out.reshape(8, 3, H, W).astype(np.float32)


# revision 25
# speedup vs baseline: 2.5088x; 1.1434x over previous
"""Trainium2 Bass kernel for nn_Entropy (histogram_binning): per-pixel Shannon
entropy of a 5x5-window KDE histogram over 256 intensity bins.

v2 design (Exp-based front end, single activation table, custom DVE ops):
  k(x,b) = sig'(10(x-b)) = u/(1+u)^2 with u = exp(-10|x-b|).
  Layout per (image): h on partitions, free = (bin-block, w-inner) with
  4 zero-pads per 100-col block (plus 5 lead / 2 trail pads per half-chunk).
  Pipeline per superchunk (8 bins x 96 w = 768 cols, 16 sc per half-pair):
    TensorE  D = x - b          (K=97 matmul: stationary [x^T; 1], moving
                                 delta-selector + (-b) row, fp32)
    DVE      a = |D - 128h|     (custom ABSDEV, fp16, strided into padded
                                 chunk; pads preset to 1e4 so exp(pad) = 0)
    ScalarE  u = Exp(-10 a)     (fp16)
    DVE      v = u + u^2(c2+c3u) (custom VPOLY ~ u/(1+u), rel err ~0.5%)
    DVE      A5 = running 5-window of k = v(1-v)  (custom WIN5K scan op:
             state += k(v[i]) - k(v[i-5]); block pads make all SAME edges
             exact with zero fixups)
    TensorE  P = band @ A5      (fp16 H-window band matmul -> PSUM = q)
    ScalarE  L = Ln(q + 2e-6)   (batched; eps-shift identity removes the
                                 per-pixel 1/S scale from the log argument)
    DVE      e = q * L          (PSUM x fp16 -> e-stripe fp16)
  Per half: 7-level pairwise tree-reduce of e over bins -> T = sum_b q ln q.
  Analytic S-path (5 taps of the same fp16 Exp/poly chain on [96,288]) gives
  S = sum_b q; E = ln(S+EPS) - T/(S+EPS).  Exp/Ln/Identity/Abs share one
  activation table -> no ACT table reloads.
  Sharding: B*C = 24 images, 3 per core across 8 cores; no collectives.
"""

import sys

sys.path.insert(0, "/opt/trn_rl_repo")

import numpy as np

H = 96
W = 96
NBH = 128          # bins per half
NIMG = 3
NCORES = 8
EPS = 1e-10
EPS1 = 2e-6
C2P, C3P = -0.89877895, 0.43582129   # v-poly coeffs: v = u + u^2(C2P + C3P u)
BLK = 100          # per-bin block: 4 pads + 96 w
NSC = 16           # superchunks per image (8 bins each, both halves)
BPS = 8            # bins per superchunk per half
HCH = 5 + BPS * BLK + 2   # half-chunk cols: 5 lead + 800 + 2 trail = 807
PAD = 1e4          # a-pad value: exp(-10*PAD) == 0

_CACHE = {}


def _register_dve_ops():
    import concourse.dve_ops as dve_ops
    from concourse.dve_ops import DveOp
    from concourse.dve_spec import (
        C0, C1, AluOp, One, Spec, Src0, Src1, maxx, scan, sq,
    )

    def register(op):
        if op.name not in dve_ops._SUB_OPCODE_FOR_NAME:
            dve_ops.OPS.append(op)
            dve_ops._SUB_OPCODE_FOR_NAME[op.name] = (
                dve_ops._CUSTOM_DVE_ROW_BASE + len(dve_ops.OPS) - 1
            )
        else:
            op = next(o for o in dve_ops.OPS if o.name == op.name)
        return op

    absdev = register(DveOp(
        "ABSDEV_ANT",
        Spec(body=maxx(Src0 - C0, C0 - Src0),
             reference=lambda in0, in1, c0, c1, c2: np.abs(
                 in0.astype(np.float32) - c0).astype(np.float32)),
        subdim=False,
        uops_sha={"v3": "a5866c869c7d6e30", "v4": "006fe4b232e6035a"}))

    vpoly = register(DveOp(
        "VPOLY_ANT",
        Spec(body=Src0 + sq(Src0) * (C0 + C1 * Src0),
             reference=lambda in0, in1, c0, c1, c2: (
                 lambda u: (u + u * u * (c0 + c1 * u)).astype(np.float32)
             )(in0.astype(np.float32))),
        subdim=False,
        perf_en={"v3": True, "v4": True},
        uops_sha={"v3": "217961e937d92645", "v4": "56741f276e7f1259"}))

    win5 = register(DveOp(
        "WIN5K_ANT",
        Spec(body=scan(AluOp.ADD, Src0 * (One - Src0) - Src1 * (One - Src1)),
             reference=lambda in0, in1, c0, c1, c2: np.cumsum(
                 in0.astype(np.float32) * (1 - in0.astype(np.float32))
                 - in1.astype(np.float32) * (1 - in1.astype(np.float32)),
                 axis=-1, dtype=np.float32)),
        subdim=False,
        perf_en={"v3": True, "v4": True},
        uops_sha={"v3": "9d91f28b1ae18abb", "v4": "1425a9f273284709"}))

    return absdev, vpoly, win5


def _patch_act_tables():
    """Force Exp and Ln onto one shared activation table so the table-load
    pass never ping-pongs between per-function tables inside the main loop."""
    import concourse.hw_specs as hw_specs
    from concourse import bacc, mybir

    if getattr(hw_specs, "_ant_act_patch", False):
        return
    AF = mybir.ActivationFunctionType
    orig = hw_specs.get_activation_tables

    def patched(arch):
        tabs = orig(arch)
        out = {}
        for name, s in tabs.items():
            if name == "natural_log_exp_and_others":
                out[name] = set(s)
            else:
                out[name] = set(s) - {AF.Exp, AF.Ln}
        return out

    hw_specs.get_activation_tables = patched
    bacc.get_activation_tables = patched
    hw_specs._ant_act_patch = True


def _build_consts():
    # selector moving operand [97, 128*96]: col c = b_local*96 + w.
    # rows k<96: delta(k == w); row 96: -b_local.
    sel = np.zeros((97, NBH * W), dtype=np.float16)
    cols = np.arange(NBH * W)
    bl = cols // W
    w = cols % W
    sel[w, cols] = 1.0
    sel[96, :] = -bl.astype(np.float16)
    hh = np.arange(H)
    band = (np.abs(hh[:, None] - hh[None, :]) <= 2).astype(np.float16)
    return sel, band


def _emit_kernel(nc, tc, ctx, ins, outs, ops):
    from concourse import mybir

    f32 = mybir.dt.float32
    f16 = mybir.dt.float16
    i32 = mybir.dt.int32
    AF = mybir.ActivationFunctionType
    OP = mybir.AluOpType

    ABSDEV, VPOLY, WIN5 = ops
    x_d, xt_d, sel_d, band_d = ins
    (ent_d,) = outs
    NW = NIMG * W

    consts = ctx.enter_context(tc.tile_pool(name="consts", bufs=1))
    sm = ctx.enter_context(tc.tile_pool(name="sm", bufs=1))
    apool = ctx.enter_context(tc.tile_pool(name="ap", bufs=2))
    upool = ctx.enter_context(tc.tile_pool(name="up", bufs=2))
    vpool = ctx.enter_context(tc.tile_pool(name="vp", bufs=2))
    a5pool = ctx.enter_context(tc.tile_pool(name="a5p", bufs=2))
    lpool = ctx.enter_context(tc.tile_pool(name="lp", bufs=2))
    epool = ctx.enter_context(tc.tile_pool(name="ep", bufs=2))
    tpool = ctx.enter_context(tc.tile_pool(name="tp", bufs=1))
    dpsum = ctx.enter_context(tc.tile_pool(name="dps", bufs=2, space="PSUM"))
    ppsum = ctx.enter_context(tc.tile_pool(name="pps", bufs=1, space="PSUM"))

    # ---- constants / inputs ----
    band_sb = consts.tile([H, H], f16)
    nc.sync.dma_start(band_sb[:], band_d[:])

    xall = consts.tile([H, NW], f32)
    xt32 = consts.tile([H, NIMG * H], f32)
    for i in range(NIMG):
        nc.sync.dma_start(xall[:, i * W:(i + 1) * W], x_d[i])
        nc.sync.dma_start(xt32[:, i * H:(i + 1) * H], xt_d[i])
    # stationaries: st_a = [round(x)^T ; ones], st_b = [frac^T ; zeros] (fp16)
    st_a = consts.tile([97, NIMG * H], f16)
    st_b = consts.tile([97, NIMG * H], f16)
    xti = consts.tile([H, NIMG * H], i32)
    nc.vector.tensor_copy(xti[:], xt32[:])
    xtif = consts.tile([H, NIMG * H], f32)
    nc.vector.tensor_copy(xtif[:], xti[:])
    nc.vector.tensor_copy(st_a[0:96, :], xtif[:])
    nc.vector.tensor_tensor(st_b[0:96, :], xt32[:], xtif[:], op=OP.subtract)
    nc.vector.memset(st_a[96:97, :], 1.0)
    nc.vector.memset(st_b[96:97, :], 0.0)

    # selector DMA'd in 16 chunks, spread over 4 queues, so the first
    # D-matmul starts early and the transfers run in parallel
    sel_sb = consts.tile([97, NBH * W], f16)
    for j in range(NSC):
        nc.gpsimd.dma_start(
            sel_sb[:, 768 * j:768 * (j + 1)], sel_d[:, 768 * j:768 * (j + 1)])

    bias_tiles = {}

    def bias_ap(val):
        if val not in bias_tiles:
            t = consts.tile([H, 1], f32, tag=f"bias{val}")
            nc.vector.memset(t[:], val)
            bias_tiles[val] = t
        return bias_tiles[val][:]

    # =====================  S path ([96, 288])  =====================
    ni = sm.tile([H, NW], i32)
    nc.vector.tensor_copy(ni[:], xall[:])
    nf = sm.tile([H, NW], f32)
    nc.vector.tensor_copy(nf[:], ni[:])
    ufrac = sm.tile([H, NW], f32)
    nc.vector.tensor_tensor(ufrac[:], xall[:], nf[:], op=OP.subtract)
    taps = (-2, -1, 0, 1, 2)
    atap = sm.tile([H, 5, NW], f16)
    for oi, o in enumerate(taps):
        # a_o = |ufrac - o|; ufrac in [-0.5, 0.5] (i32 copy rounds to nearest)
        nc.vector._custom_dve(
            ABSDEV, out=atap[:, oi, :], in0=ufrac[:], s0=float(o))
    utap = sm.tile([H, 5, NW], f16)
    nc.scalar.activation(utap[:], atap[:], AF.Exp, scale=-10.0)
    vtap = sm.tile([H, 5, NW], f16)
    nc.vector._custom_dve(VPOLY, out=vtap[:], in0=utap[:], s0=C2P, s1=C3P)
    vsq = sm.tile([H, 5, NW], f16)
    nc.vector.tensor_tensor(vsq[:], vtap[:], vtap[:], op=OP.mult)
    ktap = sm.tile([H, 5, NW], f16)
    nc.vector.tensor_tensor(ktap[:], vtap[:], vsq[:], op=OP.subtract)

    spix = sm.tile([H, NW], f32)
    nc.vector.tensor_copy(spix[:], ktap[:, 2, :])  # o=0 tap, always valid
    for oi, o in enumerate(taps):
        if o == 0:
            continue
        m = sm.tile([H, NW], f32, tag=f"m{o}")
        if o < 0:
            nc.vector.tensor_scalar(m[:], nf[:], float(-o), None, op0=OP.is_ge)
        else:
            nc.vector.tensor_scalar(
                m[:], nf[:], float(255 - o), None, op0=OP.is_le)
        tm = sm.tile([H, NW], f32, tag=f"tm{o}")
        nc.vector.tensor_tensor(tm[:], m[:], ktap[:, oi, :], op=OP.mult)
        nc.vector.tensor_tensor(spix[:], spix[:], tm[:], op=OP.add)
    spix16 = sm.tile([H, NW], f16)
    nc.vector.tensor_copy(spix16[:], spix[:])

    ps_s = ppsum.tile([H, 2048], f32, tag="pp")
    nc.tensor.matmul(ps_s[:, 0:NW], band_sb[:], spix16[:], start=True, stop=True)
    sh = sm.tile([H, NW], f32)
    nc.scalar.copy(sh[:], ps_s[:, 0:NW])
    shp = sm.tile([H, NIMG, W + 4], f32)
    nc.vector.memset(shp[:], 0.0)
    for i in range(NIMG):
        nc.vector.tensor_copy(shp[:, i, 2:2 + W], sh[:, i * W:(i + 1) * W])
    swin = sm.tile([H, NIMG, W], f32)
    nc.vector.tensor_tensor(swin[:], shp[:, :, 0:W], shp[:, :, 1:1 + W], op=OP.add)
    for j in (2, 3, 4):
        nc.vector.tensor_tensor(swin[:], swin[:], shp[:, :, j:j + W], op=OP.add)
    sw_flat = swin[:].rearrange("p a b -> p (a b)")
    rtile = sm.tile([H, NW], f32)
    nc.vector.tensor_scalar(rtile[:], sw_flat, EPS, None, op0=OP.add)
    nc.vector.reciprocal(rtile[:], rtile[:])
    lns = sm.tile([H, NW], f32)
    nc.scalar.activation(lns[:], sw_flat, AF.Ln, bias=bias_ap(EPS))

    # =====================  main path  =====================
    QL = sm.tile([H, NW], f32)
    tacc2 = sm.tile([H, 2, W], f32)

    npads_set = [0]

    for i in range(NIMG):
        nc.vector.memset(tacc2[:], 0.0)
        for sc in range(NSC):
            dt = dpsum.tile([H, 768], f32, tag="d")
            mvbase = 768 * sc
            for lo, hi in ((0, 512), (512, 768)):
                nc.tensor.matmul(
                    dt[:, lo:hi], st_a[:, i * H:(i + 1) * H],
                    sel_sb[:, mvbase + lo:mvbase + hi], start=True, stop=False)
                nc.tensor.matmul(
                    dt[:, lo:hi], st_b[:, i * H:(i + 1) * H],
                    sel_sb[:, mvbase + lo:mvbase + hi], start=False, stop=True)

            at = apool.tile([H, 2 * HCH], f16, tag="a")
            if npads_set[0] < 2:
                nc.vector.memset(at[:], PAD)
                npads_set[0] += 1
            dst0 = at[:, 5:5 + BPS * BLK] \
                .rearrange("p (b z) -> p b z", z=BLK)[:, :, 4:BLK]
            nc.vector._custom_dve(ABSDEV, out=dst0, in0=dt[:, 0:768], s0=0.0)
            dst1 = at[:, HCH + 5:HCH + 5 + BPS * BLK] \
                .rearrange("p (b z) -> p b z", z=BLK)[:, :, 4:BLK]
            nc.scalar.activation(dst1, dt[:, 0:768], AF.Abs, bias=bias_ap(-128.0))
            ut = upool.tile([H, 2 * HCH], f16, tag="u")
            nc.scalar.activation(ut[:], at[:], AF.Exp, scale=-10.0)
            vt = vpool.tile([H, 2 * HCH], f16, tag="v")
            nc.vector._custom_dve(VPOLY, out=vt[:], in0=ut[:], s0=C2P, s1=C3P)
            a5 = a5pool.tile([H, 2 * HCH], f16, tag="a5")
            nc.vector._custom_dve(
                WIN5, out=a5[:, 0:2 * HCH - 5], in0=vt[:, 5:2 * HCH],
                in1=vt[:, 0:2 * HCH - 5])

            pt = ppsum.tile([H, 2048], f32, tag="pp")
            for h in range(2):
                for pp in range(2):
                    off = h * HCH + 400 * pp + 6
                    mvap = a5[:, off:off + 400] \
                        .rearrange("p (b z) -> p b z", z=BLK)[:, :, 0:96]
                    nc.tensor.matmul(
                        pt[:, 1024 * h + 512 * pp:1024 * h + 512 * pp + 384],
                        band_sb[:], mvap, start=True, stop=True)
            lt = lpool.tile([H, 1536], f16, tag="l")
            nc.scalar.activation(
                lt[:].rearrange("p (a b) -> p a b", b=384),
                pt[:].rearrange("p (a b) -> p a b", b=512)[:, :, 0:384],
                AF.Ln, bias=bias_ap(EPS1))
            et = epool.tile([H, 2 * 1152], f16, tag="e")
            # h0: e = q*L straight from PSUM on V
            nc.vector.tensor_tensor(
                et[:, 0:768].rearrange("p (a b) -> p a b", b=384),
                pt[:, 0:1024].rearrange("p (a b) -> p a b", b=512)[:, :, 0:384],
                lt[:, 0:768].rearrange("p (a b) -> p a b", b=384),
                op=OP.mult)
            # h1: evacuate q on Scalar, multiply on Pool
            qe = epool.tile([H, 768], f16, tag="qe")
            nc.scalar.copy(
                qe[:].rearrange("p (a b) -> p a b", b=384),
                pt[:, 1024:2048].rearrange("p (a b) -> p a b", b=512)[:, :, 0:384])
            nc.gpsimd.tensor_tensor(
                et[:, 1152:1920], qe[:], lt[:, 768:1536], op=OP.mult)
            # per-sc tree over the 8 bins, both halves per instruction
            e2 = et[:].rearrange("p (a b) -> p a b", b=1152)
            nc.gpsimd.tensor_tensor(
                e2[:, :, 768:1152], e2[:, :, 0:384], e2[:, :, 384:768], op=OP.add)
            nc.gpsimd.tensor_tensor(
                e2[:, :, 0:192], e2[:, :, 768:960], e2[:, :, 960:1152], op=OP.add)
            nc.gpsimd.tensor_tensor(
                e2[:, :, 192:288], e2[:, :, 0:96], e2[:, :, 96:192], op=OP.add)
            nc.gpsimd.tensor_tensor(
                tacc2[:], tacc2[:], e2[:, :, 192:288], op=OP.add)

        nc.vector.tensor_tensor(
            QL[:, i * W:(i + 1) * W], tacc2[:, 0, :], tacc2[:, 1, :], op=OP.add)

    # E = lnS - r*T
    ent = sm.tile([H, NW], f32)
    nc.vector.tensor_tensor(ent[:], rtile[:], QL[:], op=OP.mult)
    nc.vector.tensor_tensor(ent[:], lns[:], ent[:], op=OP.subtract)
    for i in range(NIMG):
        nc.sync.dma_start(ent_d[i], ent[:, i * W:(i + 1) * W])


def _get_compiled():
    if "nc" in _CACHE:
        return _CACHE["nc"]
    from contextlib import ExitStack

    import concourse.tile as tile
    from concourse import bacc, mybir

    ops = _register_dve_ops()
    _patch_act_tables()

    f32 = mybir.dt.float32
    f16 = mybir.dt.float16
    nc = bacc.Bacc("TRN2", target_bir_lowering=False, debug=False)
    x_d = nc.dram_tensor("x_sh", [NIMG, H, W], f32, kind="ExternalInput").ap()
    xt_d = nc.dram_tensor("xt_sh", [NIMG, W, H], f32, kind="ExternalInput").ap()
    sel_d = nc.dram_tensor("sel", [97, NBH * W], f16, kind="ExternalInput").ap()
    band_d = nc.dram_tensor("band16", [H, H], f16, kind="ExternalInput").ap()
    ent_d = nc.dram_tensor("ent", [NIMG, H, W], f32, kind="ExternalOutput").ap()

    with tile.TileContext(nc) as tc:
        with ExitStack() as ctx:
            _emit_kernel(
                nc, tc, ctx, (x_d, xt_d, sel_d, band_d), (ent_d,), ops
            )
    nc.compile()
    _CACHE["nc"] = nc
    return nc


def make_in_maps(x):
    """x: full [8, 3, 96, 96] -> list of 8 per-core input dicts."""
    x = np.ascontiguousarray(np.asarray(x, dtype=np.float32))
    imgs = x.reshape(NCORES * NIMG, H, W)
    sel, band = _build_consts()
    in_maps = []
    for c in range(NCORES):
        sh = np.ascontiguousarray(imgs[c * NIMG:(c + 1) * NIMG])
        in_maps.append(
            {
                "x_sh": sh,
                "xt_sh": np.ascontiguousarray(sh.transpose(0, 2, 1)),
                "sel": sel,
                "band16": band,
            }
        )
    return in_maps


def kernel(x):
    """Full inputs in, full outputs out. x: [8, 3, 96, 96] f32."""
    from concourse.bass_utils import run_bass_kernel_spmd

    nc = _get_compiled()
    in_maps = make_in_maps(x)
    res = run_bass_kernel_spmd(nc, in_maps, list(range(NCORES)))
    out = np.stack([res.results[c]["ent"] for c in range(NCORES)])
    return out.reshape(8, 3, H, W).astype(np.float32)
